# revision 50
# baseline (speedup 1.0000x reference)
"""Transformer policy kernel for TRN2 (Bass/Tile), v3: bf16 matmuls +
fp32 residual + phase-batched software pipeline.
Verified: 2387458 ns HW, rel err 5.35e-3 (baseline 2986054 ns).

Per core (data-parallel over batch): BC=8 batches x S=256 -> T=2048 tokens.
D=512 (4 chunks), H=8 heads (HD=64), FF=2048 (16 chunks), L=8 layers.

Design (what actually survived hardware):
- Matmul operands bf16 (host-cast weights): enables Fast Weight Load
  (fp32r gets none -- it was inflating 512-col matmuls 423ns vs 215ns) and
  halves SBUF/DMA. PSUM accumulation fp32.
- Residual stream xF kept in fp32r (storing it bf16 costs ~3e-2 rel err
  over 8 layers); ln_apply writes a bf16 mirror xT via ACT as the matmul
  operand. LN stats read xF directly as fp32r (1 cyc/row at >=256 cols).
- Tableless LN rstd: Quake-seed + 1 Newton step, split DVE (int-ALU seed,
  muls; final mul writes a fresh fp32r tile -- fp32r matmul operands need a
  rounding producer) / ACT (Copy/Square live in every table). ACT table
  loads: 212 -> 18. M/R broadcast via 1-row PE matmuls (gpsimd broadcast
  straggles to 4us+).
- Per layer: [A: qkv+attn x4 segs, one-seg qkv lookahead so ACT-queued q/k
  copies never gate scores; deferred prev-layer LN2(s3) chain flushed under
  qkv(s0) PE work] then [wo+LN1 / FFN+LN2 interleaved per the emission
  order below] -- every serial LN chain hides under another segment's PE
  block. qkvo weights double-buffered; w1/w2 prefetched at layer start.
- Attention: scores auto row-packed (head pair at partition 0/64); AV at
  base partition 0 only (PSUM col-group 3 / output partitions 96:127 is a
  buggy PE quadrant -- explicit col-tiling there corrupts results); odd
  head reaches seg_o via partition-shift SBUF DMA. Denominators via ones
  matmuls; reciprocal_approx_fast + gpsimd broadcast.

Known rejected/failed directions (measured):
- fp8 DoubleRow W2: 2173915 ns but rel err 2.48e-2 > 2e-2 gate (fp8's ~3%
  per-element noise passes through dot products undiminished).
- Deferring LN2 for segs 2+3 (not just 3): NRT_EXEC_UNIT_UNRECOVERABLE.
- Ones-column-in-V denominator fold: DVE/PSUM reads need 32-aligned
  partition bases; the [1|64f] and [64f|1] layouts both dead-end.
Remaining levers: ~227us PE idle in 128-700ns dependency bubbles, ~120us
cold-clock (HAM), ~280us LDWEIGHTS/dispatch overhead vs the 1768us
column-streaming floor (currently at 74% of that roofline).
"""
import math
import contextlib
import numpy as np
import ml_dtypes

import concourse.bass as bass
import concourse.bacc as bacc
import concourse.tile as tile
from concourse import mybir

F32 = mybir.dt.float32
BF16 = mybir.dt.bfloat16
AF = mybir.ActivationFunctionType
ALU = mybir.AluOpType

BF_NP = ml_dtypes.bfloat16

BC = 8
S = 256
T = BC * S
OBS = 96
ACT_DIM = 29
D = 512
H = 8
HD = 64
FF = 2048
NC_D = D // 128
NC_FF = FF // 128
TT = 512
NSEG = T // TT
EPS = 1e-5
L_MAX = 8


def _nz(a):
    return a is not None and bool(np.any(np.asarray(a) != 0))


def _ng(a):
    return a is not None and bool(np.any(np.asarray(a) != 1))


def build(inputs, n_layers=8, emit_head=True, dbg_x=False):
    """inputs: dict of full np arrays (reference naming). Returns (nc, extra_in_map)."""
    nc = bacc.Bacc("TRN2", target_bir_lowering=False, debug=False)

    flags = dict(
        bin_=_nz(inputs["b_in"]), gin=_ng(inputs["g_in"]), bein=_nz(inputs["be_in"]),
        bq=_nz(inputs["bq"]), bk=_nz(inputs["bk"]), bv=_nz(inputs["bv"]), bo=_nz(inputs["bo"]),
        g1=_ng(inputs["g1"]), be1=_nz(inputs["be1"]), b1=_nz(inputs["b1"]), b2=_nz(inputs["b2"]),
        g2=_ng(inputs["g2"]), be2=_nz(inputs["be2"]),
        bp1=_nz(inputs["bp1"]), gp1=_ng(inputs["gp1"]), bep1=_nz(inputs["bep1"]),
        bp2=_nz(inputs["bp2"]), gp2=_ng(inputs["gp2"]), bep2=_nz(inputs["bep2"]),
        bp3=_nz(inputs["bp3"]), asc=_ng(inputs["action_scale"]), abi=_nz(inputs["action_bias"]),
    )

    def din(name, shape, dt=BF16):
        return nc.dram_tensor(name, shape, dt, kind="ExternalInput").ap()

    F32R = mybir.dt.float32r
    obs_d = din("observations", (BC, S, OBS))
    win_d = din("W_in", (OBS, D))
    wq_d = din("Wq", (L_MAX, D, D)); wk_d = din("Wk", (L_MAX, D, D))
    wv_d = din("Wv", (L_MAX, D, D)); wo_d = din("Wo", (L_MAX, D, D))
    w1_d = din("W1", (L_MAX, D, FF)); w2_d = din("W2", (L_MAX, FF, D))
    wp1_d = din("Wp1", (D, D // 2)); wp2_d = din("Wp2", (D // 2, D // 4))
    wp3_d = din("Wp3", (D // 4, ACT_DIM))
    identb_d = din("IDENTB", (128, 128))
    identf_d = din("IDENTF", (128, 128), F32)
    ones_d = din("ONES", (128, 8))
    onesf_d = din("ONESF", (128, 128), F32R)
    pet_d = din("PET", (D, S))
    out_d = nc.dram_tensor("OUT", (T, ACT_DIM), F32, kind="ExternalOutput").ap()
    if dbg_x:
        xdbg_d = nc.dram_tensor("XDBG", (D, T), BF16, kind="ExternalOutput").ap()

    extra = {
        "IDENTB": np.eye(128, dtype=BF_NP),
        "IDENTF": np.eye(128, dtype=np.float32),
        "ONES": np.ones((128, 8), BF_NP),
        "ONESF": np.ones((128, 128), np.float32),
    }
    pos = np.arange(S, dtype=np.float32)[:, None]
    div = np.exp(np.arange(0, D, 2, dtype=np.float32) * (-math.log(10000.0) / D))
    pe = np.zeros((S, D), dtype=np.float32)
    pe[:, 0::2] = np.sin(pos * div)
    pe[:, 1::2] = np.cos(pos * div)
    extra["PET"] = np.ascontiguousarray(pe.T).astype(BF_NP)
    for k in ["W_in", "Wq", "Wk", "Wv", "Wo", "W1", "W2", "Wp1", "Wp2", "Wp3"]:
        extra[k] = np.ascontiguousarray(np.asarray(inputs[k], np.float32)).astype(BF_NP)

    # per-feature bias/gain vectors (feature-major [128, n]) -- only emitted
    # when the corresponding values are nontrivial (not for this problem).
    def vec_tensor(name, arr):
        a = np.asarray(arr, np.float32).reshape(-1)
        n = a.size // 128
        extra[name] = np.ascontiguousarray(a.reshape(n, 128).T)
        return din(name, (128, n), F32)

    dv = {}
    for key, nm in [("bq", "BQ"), ("bk", "BK"), ("bo", "BO"), ("b1", "B1"), ("b2", "B2"),
                    ("b_in", "BIN"), ("g_in", "GIN"), ("be_in", "BEIN"),
                    ("g1", "G1"), ("be1", "BE1"), ("g2", "G2"), ("be2", "BE2"),
                    ("bp1", "BP1"), ("gp1", "GP1"), ("bep1", "BEP1"),
                    ("bp2", "BP2"), ("gp2", "GP2"), ("bep2", "BEP2")]:
        fkey = {"b_in": "bin_", "g_in": "gin", "be_in": "bein"}.get(key, key)
        if flags[fkey]:
            dv[nm] = vec_tensor(nm + "v", inputs[key])
    if flags["bv"]:
        extra["BVr"] = np.asarray(inputs["bv"], np.float32).astype(BF_NP).reshape(L_MAX, D)
        dv["BV"] = din("BVr", (L_MAX, D))

    def vec29(name, arr):
        a = np.zeros((128, 1), np.float32)
        a[:ACT_DIM, 0] = np.asarray(arr, np.float32).reshape(-1)
        extra[name] = a
        return din(name, (128, 1), F32)
    if flags["bp3"]:
        dv["BP3"] = vec29("BP3v", inputs["bp3"])
    if flags["asc"]:
        dv["ASC"] = vec29("ASCv", inputs["action_scale"])
    if flags["abi"]:
        dv["ABI"] = vec29("ABIv", inputs["action_bias"])

    scale = 1.0 / math.sqrt(HD)

    with tile.TileContext(nc) as tc:
        with contextlib.ExitStack() as ctx:
            P = {}
            P["persist"] = ctx.enter_context(tc.tile_pool(name="persist", bufs=1))
            P["wpool"] = ctx.enter_context(tc.tile_pool(name="wpool", bufs=1))
            P["xpool"] = ctx.enter_context(tc.tile_pool(name="xpool", bufs=1))
            P["segt"] = ctx.enter_context(tc.tile_pool(name="segt", bufs=1))
            P["hpool"] = ctx.enter_context(tc.tile_pool(name="hpool", bufs=1))
            P["exps"] = ctx.enter_context(tc.tile_pool(name="exps", bufs=2))
            P["sq"] = ctx.enter_context(tc.tile_pool(name="sq", bufs=2))
            P["scratch"] = ctx.enter_context(tc.tile_pool(name="scratch", bufs=2))
            P["mini"] = ctx.enter_context(tc.tile_pool(name="mini", bufs=1))
            P["rbs"] = ctx.enter_context(tc.tile_pool(name="rbs", bufs=2))
            P["rec"] = ctx.enter_context(tc.tile_pool(name="rec", bufs=1))
            P["ppro"] = ctx.enter_context(tc.tile_pool(name="ppro", bufs=2, space="PSUM"))
            P["pacc"] = ctx.enter_context(tc.tile_pool(name="pacc", bufs=2, space="PSUM"))
            P["psmall"] = ctx.enter_context(tc.tile_pool(name="psmall", bufs=4, space="PSUM"))

            # ---------------- constants ----------------
            identb = P["persist"].tile([128, 128], BF16, tag="identb")
            nc.sync.dma_start(out=identb, in_=identb_d[:, :])
            identf = P["persist"].tile([128, 128], F32, tag="identf")
            nc.sync.dma_start(out=identf, in_=identf_d[:, :])
            ones = P["persist"].tile([128, 8], BF16, tag="ones")
            nc.sync.dma_start(out=ones, in_=ones_d[:, :])
            ones_col = ones[:, 0:1]
            F32R = mybir.dt.float32r
            onesf = P["persist"].tile([128, 128], F32R, tag="onesf")
            nc.sync.dma_start(out=onesf, in_=onesf_d[:, :])
            onesf_col = onesf[:, 0:1]
            onesf_row = onesf[0:1, :]

            peT = P["persist"].tile([128, NC_D * S], BF16, tag="peT")
            nc.sync.dma_start(out=peT.rearrange("p (c s) -> p c s", s=S),
                              in_=pet_d.rearrange("(c p) s -> p c s", p=128))

            vt = {}
            for nm, d in dv.items():
                if nm == "BV":
                    t = P["persist"].tile([1, L_MAX * D], BF16, tag="c_BV")
                    for l in range(L_MAX):
                        nc.sync.dma_start(out=t[:, l * D:(l + 1) * D], in_=d[l:l + 1, :])
                else:
                    t = P["persist"].tile([128, d.shape[1]], F32, tag=f"c_{nm}")
                    nc.sync.dma_start(out=t, in_=d[:, :])
                vt[nm] = t

            def ap_vec(nm, idx):
                t = vt.get(nm)
                return t[:, idx:idx + 1] if t is not None else None

            # ---------------- big tiles ----------------
            # xF: fp32 residual stream (rounding the residual to bf16 costs
            # ~3e-2 rel err over 8 layers); xT: bf16 mirror fed to matmuls.
            xF = [[P["xpool"].tile([128, TT], F32R, tag=f"xF{c}_{s}", name=f"xF{c}_{s}")
                   for s in range(NSEG)] for c in range(NC_D)]
            xT = [[P["xpool"].tile([128, TT], BF16, tag=f"xT{c}_{s}", name=f"xT{c}_{s}")
                   for s in range(NSEG)] for c in range(NC_D)]
            # double-buffered q/k/v (indexed seg%2), per-seg o
            seg_q = [[P["segt"].tile([128, TT], BF16, tag=f"sq{c}_{b}", name=f"sq{c}_{b}")
                      for c in range(NC_D)] for b in range(2)]
            seg_k = [[P["segt"].tile([128, TT], BF16, tag=f"sk{c}_{b}", name=f"sk{c}_{b}")
                      for c in range(NC_D)] for b in range(2)]
            # seg_v carries 8 heads x (64 feat + 1 ones col): the ones col
            # folds the softmax denominator into the AV matmul (row 64 of
            # the [65, S] AV output = sum of exp), killing the aux ones
            # matmuls (4 per pair, ~112us of PE across the kernel).
            seg_v = [[P["segt"].tile([128, H * (HD + 1)], BF16, tag=f"sv{c}_{b}",
                                     name=f"sv{c}_{b}")
                      for c in range(NC_D)] for b in range(2)]
            seg_o = [[P["segt"].tile([128, TT], BF16, tag=f"so{c}_{s}", name=f"so{c}_{s}")
                      for c in range(NC_D)] for s in range(NSEG)]
            hT = [P["hpool"].tile([128, TT], BF16, tag=f"hT{m}", name=f"hT{m}")
                  for m in range(NC_FF)]

            # weights: qkvo+w1 double-buffered, w2 single (prefetched per layer)
            # each matrix is ONE [128, NC*D] tile (chunk-major cols) so a
            # whole matrix loads in a single DMA: the sync queue costs
            # ~610ns per dma_start, so 36 small DMAs/layer was ~20us of
            # issue serialization.
            wq_b = [P["wpool"].tile([128, NC_D * D], BF16, tag=f"wq_{b}", name=f"wq_{b}")
                    for b in range(2)]
            wk_b = [P["wpool"].tile([128, NC_D * D], BF16, tag=f"wk_{b}", name=f"wk_{b}")
                    for b in range(2)]
            wv_b = [P["wpool"].tile([128, NC_D * D], BF16, tag=f"wv_{b}", name=f"wv_{b}")
                    for b in range(2)]
            wo_b = [P["wpool"].tile([128, NC_D * D], BF16, tag=f"wo_{b}", name=f"wo_{b}")
                    for b in range(2)]
            wq_t = [[wq_b[b][:, k * D:(k + 1) * D] for k in range(NC_D)] for b in range(2)]
            wk_t = [[wk_b[b][:, k * D:(k + 1) * D] for k in range(NC_D)] for b in range(2)]
            wv_t = [[wv_b[b][:, k * D:(k + 1) * D] for k in range(NC_D)] for b in range(2)]
            wo_t = [[wo_b[b][:, k * D:(k + 1) * D] for k in range(NC_D)] for b in range(2)]
            w1_bt = P["wpool"].tile([128, NC_D * FF], BF16, tag="w1", name="w1t")
            w1_t = [w1_bt[:, k * FF:(k + 1) * FF] for k in range(NC_D)]
            w2_bt = P["wpool"].tile([128, NC_FF * D], BF16, tag="w2", name="w2t")
            w2_t = [w2_bt[:, k * D:(k + 1) * D] for k in range(NC_FF)]
            # head/input weights: win aliases w2_t[0] (w2 of layer 0 is
            # loaded only after the input-stage projections are emitted)
            win_t = w2_t[0][:, :]
            wp1_t = [P["wpool"].tile([128, D // 2], BF16, tag=f"wp1{k}", name=f"wp1{k}")
                     for k in range(NC_D)]
            wp2_t = [P["wpool"].tile([128, D // 4], BF16, tag=f"wp2{k}", name=f"wp2{k}")
                     for k in range(2)]
            wp3_t = P["wpool"].tile([128, 32], BF16, tag="wp3")

            def load_layer_weights(lx, b):
                for dst, srcd in [(wq_b[b], wq_d), (wk_b[b], wk_d),
                                   (wv_b[b], wv_d), (wo_b[b], wo_d)]:
                    nc.sync.dma_start(
                        out=dst.rearrange("p (k j) -> p k j", j=D),
                        in_=srcd[lx].rearrange("(k p) j -> p k j", p=128))

            def load_w12(lx):
                nc.sync.dma_start(out=w1_bt.rearrange("p (k j) -> p k j", j=FF),
                                  in_=w1_d[lx].rearrange("(k p) j -> p k j", p=128))
                nc.sync.dma_start(out=w2_bt.rearrange("p (k j) -> p k j", j=D),
                                  in_=w2_d[lx].rearrange("(k p) j -> p k j", p=128))

            # ---------------- layernorm (split into stats / chain / apply) ----
            def mk_sq(chunk_ap):
                sqt = P["sq"].tile([128, TT], F32R, tag="sqt")
                nc.scalar.square(sqt, chunk_ap)
                return sqt

            def ln_stats(chunk_aps, nfeat, sqts=None):
                """stats over fp32r chunks: squares on ACT, sums via ones
                matmuls (fp32r, 1 cyc/row at >=256 cols). All sums matmuls
                go first so the PE isn't gated on ACT square latency."""
                nch = len(chunk_aps)
                sums = P["psmall"].tile([1, TT], F32, tag="sm", name="sums")
                sumsq = P["psmall"].tile([1, TT], F32, tag="sm", name="sumsq")
                if sqts is None:
                    sqts = [mk_sq(chunk_aps[c]) for c in range(nch)]
                for c in range(nch):
                    nc.tensor.matmul(sums, onesf_col, chunk_aps[c],
                                     start=(c == 0), stop=(c == nch - 1))
                for c in range(nch):
                    nc.tensor.matmul(sumsq, onesf_col, sqts[c],
                                     start=(c == 0), stop=(c == nch - 1))
                return sums, sumsq

            def ln_chain_pre(stats, nfeat, n_newton=1):
                """mean + rstd for a token-column LN, tableless: rstd via
                Quake-seed + Newton, split across ACT (Copy/Square, in every
                table) and DVE (muls + int seed). Returns (m, yr) row
                vectors; ln_chain_bc broadcasts them on the PE."""
                sums, sumsq = stats
                I32 = mybir.dt.int32
                m = P["mini"].tile([1, TT], F32R, tag="m", bufs=2)
                nc.scalar.mul(m, sums, 1.0 / nfeat)
                e2 = P["mini"].tile([1, TT], F32, tag="e2")
                nc.scalar.activation(e2, sumsq, AF.Copy, bias=float(EPS),
                                     scale=1.0 / nfeat)
                msq = P["mini"].tile([1, TT], F32, tag="msq")
                nc.scalar.square(msq, m)
                nc.vector.tensor_sub(e2, e2, msq)  # e2 = var + eps
                y = P["mini"].tile([1, TT], F32, tag="y")
                nc.vector.tensor_scalar(out=y.bitcast(I32), in0=e2.bitcast(I32),
                                        scalar1=1, scalar2=None,
                                        op0=ALU.logical_shift_right)
                nc.vector.tensor_scalar(out=y.bitcast(I32), in0=y.bitcast(I32),
                                        scalar1=0x5F3759DF, scalar2=-1,
                                        op0=ALU.subtract, op1=ALU.mult)
                # last Newton step writes a fresh fp32r tile: fp32r matmul
                # operands must come from a producer that rounds to fp32r,
                # which the int-ALU seed ops above don't.
                yr = P["mini"].tile([1, TT], F32R, tag="yr", bufs=2)
                for it in range(n_newton):
                    a = P["mini"].tile([1, TT], F32, tag="nta")
                    nc.vector.tensor_mul(a, y, y)
                    nc.vector.tensor_mul(a, a, e2)
                    nc.scalar.activation(a, a, AF.Copy, bias=1.5, scale=-0.5)
                    nc.vector.tensor_mul(yr if it == n_newton - 1 else y, y, a)
                return m, yr

            def ln_chain_bc(pre):
                # broadcast M/R across partitions on the PE (gpsimd broadcast
                # latency straggles to 4us+ and stalled the FFN start). The
                # ones stationary row is picked at the moving operand's base
                # partition (matmul requires matching bases).
                m, yr = pre
                bm, br = m.base_partition(), yr.base_partition()
                M = P["pacc"].tile([128, TT], F32, tag="pa", name="Mb")
                nc.tensor.matmul(M, onesf[bm:bm + 1, :], m, start=True, stop=True)
                R = P["pacc"].tile([128, TT], F32, tag="pa", name="Rb")
                nc.tensor.matmul(R, onesf[br:br + 1, :], yr, start=True, stop=True)
                return M, R

            def ln_chain(stats, nfeat, n_newton=1):
                return ln_chain_bc(ln_chain_pre(stats, nfeat, n_newton))

            def ln_apply(bc, chunk_aps, g_fn=None, b_fn=None, gelu=False,
                         bf_out=None):
                """normalize fp32r chunks in place; optionally gelu; optionally
                write a bf16 mirror (the matmul operand) via ACT."""
                M, R = bc
                for c in range(len(chunk_aps)):
                    xc = chunk_aps[c]
                    g_ap = g_fn(c) if g_fn is not None else None
                    b_ap = b_fn(c) if b_fn is not None else None
                    nc.vector.tensor_sub(xc, xc, M)
                    if g_ap is not None:
                        nc.vector.scalar_tensor_tensor(xc, xc, g_ap, R,
                                                       ALU.mult, ALU.mult)
                    else:
                        nc.vector.tensor_mul(xc, xc, R)
                    if gelu:
                        nc.scalar.activation(xc, xc, AF.Gelu,
                                             bias=b_ap if b_ap is not None else 0.0,
                                             scale=1.0)
                    elif b_ap is not None:
                        nc.scalar.activation(xc, xc, AF.Identity, bias=b_ap,
                                             scale=1.0)
                    if bf_out is not None:
                        nc.scalar.copy(bf_out[c], xc)

            # ---------------- projections ----------------
            def proj_fm(w_tiles, in_aps, out_aps, bias_fn=None, kpart=128,
                        epi="act"):
                """feature-major projection: out[mc] = W.T @ in (+bias).
                Epilogue copy on ACT by default (DVE is the busier engine)."""
                n_out = len(out_aps)
                n_in = len(in_aps)
                for mc in range(n_out):
                    ps = P["ppro"].tile([128, TT], F32, tag="pp")
                    for kc in range(n_in):
                        nc.tensor.matmul(
                            ps, w_tiles[kc][0:kpart, mc * 128:(mc + 1) * 128],
                            in_aps[kc][0:kpart, :],
                            start=(kc == 0), stop=(kc == n_in - 1))
                    b_ap = bias_fn(mc) if bias_fn is not None else None
                    if b_ap is None and epi == "dve":
                        nc.vector.tensor_copy(out_aps[mc], ps)
                    elif b_ap is None:
                        nc.scalar.copy(out_aps[mc], ps)
                    else:
                        nc.scalar.activation(out_aps[mc], ps, AF.Identity,
                                             bias=b_ap, scale=1.0)

            # ---------------- attention ----------------
            def qkv_chunks(seg, l, wb):
                """return the q/k/v projection of one seg as 12 single-psum
                chunk callables: [q0,k0,q1,k1,q2,k2] (safe to inject into
                the PREVIOUS seg-parity attention at pair slots 2.. with
                c-major pair order) and [q3,k3,v0..v3] (safe once that
                attention is fully emitted)."""
                db = seg % 2
                xs = [xT[c][seg][:, :] for c in range(NC_D)]

                def q_chunk(mc):
                    def f():
                        ps = P["ppro"].tile([128, TT], F32, tag="pp")
                        for kc in range(NC_D):
                            nc.tensor.matmul(
                                ps, wq_t[wb][kc][:, mc * 128:(mc + 1) * 128],
                                xs[kc], start=(kc == 0), stop=(kc == NC_D - 1))
                        b_ap = ap_vec("BQ", l * 4 + mc) if flags["bq"] else None
                        if b_ap is None:
                            nc.scalar.copy(seg_q[db][mc][:, :], ps)
                        else:
                            nc.scalar.activation(seg_q[db][mc][:, :], ps,
                                                 AF.Identity, bias=b_ap, scale=1.0)
                    return f

                def k_chunk(mc):
                    def f():
                        ps = P["ppro"].tile([128, TT], F32, tag="pp")
                        for kc in range(NC_D):
                            nc.tensor.matmul(
                                ps, wk_t[wb][kc][:, mc * 128:(mc + 1) * 128],
                                xs[kc], start=(kc == 0), stop=(kc == NC_D - 1))
                        b_ap = ap_vec("BK", l * 4 + mc) if flags["bk"] else None
                        if b_ap is None:
                            nc.vector.tensor_copy(seg_k[db][mc][:, :], ps)
                        else:
                            nc.scalar.activation(seg_k[db][mc][:, :], ps,
                                                 AF.Identity, bias=b_ap, scale=1.0)
                    return f

                def v_chunk(ts):
                    def f():
                        vp = P["ppro"].tile([128, D], F32, tag="pp")
                        for kc in range(NC_D):
                            nc.tensor.matmul(
                                vp, xT[kc][seg][:, ts * 128:(ts + 1) * 128],
                                wv_t[wb][kc],
                                start=(kc == 0), stop=(kc == NC_D - 1) and not flags["bv"])
                        if flags["bv"]:
                            nc.tensor.matmul(vp, ones[0:1, 0:128],
                                             vt["BV"][:, l * D:(l + 1) * D],
                                             start=False, stop=True)
                        nc.vector.tensor_copy(
                            seg_v[db][ts].rearrange("p (h g) -> p h g", g=HD + 1)[:, :, 0:HD],
                            vp.rearrange("p (h f) -> p h f", f=HD))
                    return f

                early = [q_chunk(0), k_chunk(0), q_chunk(1), k_chunk(1),
                         q_chunk(2), k_chunk(2)]
                late = [q_chunk(3), k_chunk(3)] + [v_chunk(ts) for ts in range(4)]
                return early, late

            def emit_qkv(seg, l, wb):
                early, late = qkv_chunks(seg, l, wb)
                for f in early + late:
                    f()

            def emit_attn(seg, inject=None):
                """softmax attention for one segment (2 batches x 4 head pairs).
                One-pair lookahead: pair p+1's score matmuls are emitted before
                pair p's AV matmuls so the PE never waits on Exp. Pair order is
                c-major so seg_q/k chunk c has no readers after pair 2c+1 --
                lets the next-parity qkv chunks inject early. `inject` is a
                list of callables emitting ready PE work, consumed one per
                pair slot to fill the exp-latency bubbles."""
                db = seg % 2
                pairs = [(b2, c) for c in range(NC_D) for b2 in range(2)]
                inject = list(inject) if inject else []
                st = {}

                def do_inject(n=1):
                    for _ in range(n):
                        if inject:
                            inject.pop(0)()

                def scores(p):
                    b2, c = pairs[p]
                    bcol = b2 * S
                    scps, ess = [], []
                    for hh in range(2):
                        roff = hh * HD
                        scp = P["psmall"].tile([128, 2 * S], F32, tag="sm",
                                               name=f"scp{hh}")
                        for kc in range(2):
                            nc.tensor.matmul(
                                scp[:, kc * S:(kc + 1) * S],
                                seg_k[db][c][roff:roff + HD,
                                             bcol + kc * 128: bcol + (kc + 1) * 128],
                                seg_q[db][c][roff:roff + HD, bcol:bcol + S],
                                start=True, stop=True)
                        scps.append(scp)
                    for hh in range(2):
                        esh = P["exps"].tile([128, 2 * S], BF16, tag=f"es{hh}",
                                             name=f"es{hh}")
                        nc.scalar.activation(esh, scps[hh], AF.Exp, bias=0.0,
                                             scale=scale)
                        ess.append(esh)
                    st[p] = ess

                def avpart(p):
                    # v1-proven AV layout: both heads' AV outputs at base
                    # partition 0 (PSUM col-group 3 -- output partitions
                    # 96:127 -- is a buggy PE quadrant, so no col-tiling);
                    # the odd head reaches seg_o partitions 64:128 via an
                    # SBUF->SBUF partition-shift DMA. aux+otp are 2 "sm"
                    # slots (not 3) so the 4-slot round-robin never makes
                    # pair p's AV matmuls wait on pair p+1's exp.
                    b2, c = pairs[p]
                    bcol = b2 * S
                    ess = st.pop(p)
                    otpb = P["psmall"].tile([HD + 1, 2 * S], F32, tag="sm", name="otpb")
                    otp = [otpb[0:HD, hh * S:(hh + 1) * S] for hh in range(2)]
                    for hh in range(2):
                        h = 2 * c + hh
                        for kc in range(2):
                            nc.tensor.matmul(
                                otpb[0:HD + 1, hh * S:(hh + 1) * S],
                                seg_v[db][b2 * 2 + kc][:, h * (HD + 1):(h + 1) * (HD + 1)],
                                ess[hh][:, kc * S:(kc + 1) * S],
                                start=(kc == 0), stop=(kc == 1))
                    # denom row 64 of the AV psum: DVE reads at partition
                    # base 64 of PSUM return garbage (HW quirk) and gpsimd
                    # can't read PSUM at all, so a sync DMA ferries the row
                    # to SBUF (ACT is the attention-phase bottleneck); recip
                    # runs 128-lane-wide after the broadcast. One wide mul
                    # divides both heads; one 3D-AP DMA scatters the halves
                    # into seg_o partitions 0:64 / 64:128.
                    rec = P["rec"].tile([1, 2 * S], F32, tag="rec")
                    nc.scalar.copy(rec, otpb[HD:HD + 1, :])
                    rbs = P["rbs"].tile([128, 2 * S], F32, tag="rbs")
                    nc.gpsimd.partition_broadcast(rbs, rec)
                    nc.vector.reciprocal_approx_fast(out=rbs, in_=rbs)
                    nc.vector.tensor_mul(
                        seg_o[seg][c][0:HD, bcol:bcol + S],
                        otp[0], rbs[0:HD, 0:S])
                    otmp = P["rbs"].tile([HD, S], BF16, tag="otmp")
                    nc.vector.tensor_mul(otmp, otp[1], rbs[0:HD, S:2 * S])
                    nc.sync.dma_start(out=seg_o[seg][c][HD:128, bcol:bcol + S],
                                      in_=otmp)

                scores(0)
                for p in range(1, 8):
                    scores(p)
                    avpart(p - 1)
                    # pair p's q/k chunk frees at pair 2c+1; injected chunk
                    # c's epilogue waits scores(2c+1), so keep c <= (p-2)//2
                    # to avoid parking the ACT/DVE queue on a long wait.
                    do_inject(1)
                avpart(7)
                do_inject(len(inject))

            # ---------------- input stage ----------------
            # ---- PE warm-up: ~4.5us of back-to-back matmuls releases the HAM
            # clock throttle (cold PE runs at 1.2 instead of 2.4 GHz) before
            # the thin-PE input stage and layer 0 begin.
            warm_ps = P["ppro"].tile([128, 128], F32, tag="pp", name="warmps")
            for _ in range(40):
                nc.tensor.matmul(warm_ps, identb, identb, start=True, stop=True)
            warm_out = P["scratch"].tile([128, 128], BF16, tag="warmo")
            nc.vector.tensor_copy(warm_out, warm_ps)

            # obs DMAs batched (1/seg) and issued before the weight loads so
            # the transposes aren't stuck behind ~16us of sync-queue issue.
            obs_flat = obs_d.rearrange("b s f -> (b s) f")
            ot_segs = []
            for seg in range(NSEG):
                # stage in dead seg_v tiles (free until layer-0 qkv)
                ot = seg_v[seg % 2][seg // 2][:, 0:4 * OBS]
                nc.sync.dma_start(
                    out=ot.rearrange("p (ts f) -> p ts f", f=OBS),
                    in_=obs_flat[seg * TT:(seg + 1) * TT]
                        .rearrange("(ts p) f -> p ts f", p=128))
                ot_segs.append(ot)
            nc.sync.dma_start(out=win_t[0:OBS, :], in_=win_d[:, :])
            in_stats = {}
            for seg in range(NSEG):
                obsT = seg_k[seg % 2][seg // 2]  # [96, 512] region staging
                for ts in range(4):
                    tp = P["psmall"].tile([OBS, 128], BF16, tag="sm", name="tpin")
                    nc.tensor.transpose(
                        tp, ot_segs[seg][:, ts * OBS:(ts + 1) * OBS], identb)
                    nc.vector.tensor_copy(obsT[0:OBS, ts * 128:(ts + 1) * 128], tp)
                xf = [xF[c][seg][:, :] for c in range(NC_D)]
                proj_fm([win_t], [obsT[:, :]], xf,
                        (lambda mc: ap_vec("BIN", mc)) if flags["bin_"] else None,
                        kpart=OBS)
                in_stats[seg] = ln_stats(xf, D)
            load_layer_weights(0, 0)
            load_w12(0)  # w2_t[0] aliases win; DMA waits the proj reads above
            # ones columns of seg_v (col h*65+64): written once, after the
            # obs staging reads; v-projection epilogues never touch them.
            for b in range(2):
                for ts in range(4):
                    nc.vector.tensor_copy(
                        seg_v[b][ts].rearrange("p (h g) -> p h g", g=HD + 1)[:, :, HD:HD + 1],
                        ones[:, 0:H].unsqueeze(2))
            in_bc = {}
            for seg in range(NSEG):
                in_bc[seg] = ln_chain(in_stats[seg], D)
            for seg in range(NSEG):
                xf = [xF[c][seg][:, :] for c in range(NC_D)]
                ln_apply(in_bc[seg], xf,
                         (lambda c: ap_vec("GIN", c)) if flags["gin"] else None,
                         (lambda c: ap_vec("BEIN", c)) if flags["bein"] else None,
                         gelu=True)
                for c in range(NC_D):
                    xc = xf[c]
                    nc.vector.tensor_add(
                        xc.rearrange("p (b s) -> p b s", s=S),
                        xc.rearrange("p (b s) -> p b s", s=S),
                        peT[:, c * S:(c + 1) * S].unsqueeze(1)
                           .broadcast_to([128, TT // S, S]))
                    nc.scalar.copy(xT[c][seg][:, :], xc)

            # ---------------- layers ----------------
            # ln2 chain of the last couple of segs is deferred into the next
            # layer's attention phase so it hides under qkv/attention PE work
            # instead of stalling the layer tail.
            pending_ln2 = []

            def flush_ln2(l_prev):
                for seg, st in pending_ln2:
                    xf = [xF[c][seg][:, :] for c in range(NC_D)]
                    bc = ln_chain(st, D)
                    ln_apply(bc, xf,
                             (lambda c: ap_vec("G2", l_prev * 4 + c)) if flags["g2"] else None,
                             (lambda c: ap_vec("BE2", l_prev * 4 + c)) if flags["be2"] else None,
                             bf_out=[xT[c][seg][:, :] for c in range(NC_D)])
                pending_ln2.clear()

            # ---------------- head helpers (per-seg, interleaved into the
            # last layer's FFN blocks so the LN chains hide under PE work
            # and the PE never goes cold at the kernel tail) --------------
            def hp1(seg):
                xs = [xT[c][seg][:, :] for c in range(NC_D)]
                y1f = [xF[mc][seg][:, :] for mc in range(2)]
                proj_fm(wp1_t, xs, y1f,
                        (lambda mc: ap_vec("BP1", mc)) if flags["bp1"] else None)
                return ln_stats(y1f, D // 2)

            def apply1(seg, bc):
                y1f = [xF[mc][seg][:, :] for mc in range(2)]
                y1b = [seg_q[seg % 2][mc][:, :] for mc in range(2)]
                ln_apply(bc, y1f,
                         (lambda c: ap_vec("GP1", c)) if flags["gp1"] else None,
                         (lambda c: ap_vec("BEP1", c)) if flags["bep1"] else None,
                         gelu=True, bf_out=y1b)

            def hp2(seg):
                y1b = [seg_q[seg % 2][mc][:, :] for mc in range(2)]
                y2f = [xF[2][seg][:, :]]
                proj_fm(wp2_t, y1b, y2f,
                        (lambda mc: ap_vec("BP2", 0)) if flags["bp2"] else None)
                return ln_stats(y2f, D // 4)

            def apply2(seg, bc):
                y2f = [xF[2][seg][:, :]]
                y2b = [seg_k[seg % 2][0][:, :]]
                ln_apply(bc, y2f,
                         (lambda c: ap_vec("GP2", 0)) if flags["gp2"] else None,
                         (lambda c: ap_vec("BEP2", 0)) if flags["bep2"] else None,
                         gelu=True, bf_out=y2b)

            def hp3(seg):
                y2b = seg_k[seg % 2][0][:, :]
                actp = P["psmall"].tile([ACT_DIM, TT], F32, tag="sm", name="actp")
                nc.tensor.matmul(actp, wp3_t[:, 0:ACT_DIM], y2b,
                                 start=True, stop=True)
                actT = P["mini"].tile([ACT_DIM, TT], F32, tag="actT")
                nc.scalar.activation(actT[0:ACT_DIM, :], actp, AF.Tanh,
                                     bias=vt["BP3"][0:ACT_DIM, 0:1] if flags["bp3"] else 0.0,
                                     scale=1.0)
                if flags["asc"] or flags["abi"]:
                    nc.scalar.activation(
                        actT[0:ACT_DIM, :], actT[0:ACT_DIM, :], AF.Identity,
                        bias=vt["ABI"][0:ACT_DIM, 0:1] if flags["abi"] else 0.0,
                        scale=vt["ASC"][0:ACT_DIM, 0:1] if flags["asc"] else 1.0)
                for ts in range(4):
                    tp = P["ppro"].tile([128, ACT_DIM], F32, tag="pp", name="tpo")
                    nc.tensor.transpose(tp, actT[0:ACT_DIM, ts * 128:(ts + 1) * 128],
                                        identf[0:ACT_DIM, 0:ACT_DIM])
                    ob = P["scratch"].tile([128, ACT_DIM], F32, tag="ob")
                    nc.vector.tensor_copy(ob, tp)
                    nc.sync.dma_start(
                        out=out_d[seg * TT + ts * 128: seg * TT + (ts + 1) * 128, :],
                        in_=ob)

            def load_head_weights():
                for k in range(NC_D):
                    nc.sync.dma_start(out=wp1_t[k], in_=wp1_d[k * 128:(k + 1) * 128, :])
                for k in range(2):
                    nc.sync.dma_start(out=wp2_t[k], in_=wp2_d[k * 128:(k + 1) * 128, :])
                nc.sync.dma_start(out=wp3_t[:, 0:ACT_DIM], in_=wp3_d[:, :])

            for l in range(n_layers):
                wb = l % 2
                if l + 1 < n_layers:
                    load_layer_weights(l + 1, 1 - wb)
                if l > 0:
                    load_w12(l)
                if emit_head and l == n_layers - 1:
                    load_head_weights()

                # phase B helpers (defined first; phase A injects wo chunks)
                def wo_chunks(seg):
                    xf = [xF[c][seg][:, :] for c in range(NC_D)]

                    def chunk(mc):
                        def f():
                            ps = P["ppro"].tile([128, TT], F32, tag="pp")
                            for kc in range(NC_D):
                                nc.tensor.matmul(
                                    ps, wo_t[wb][kc][:, mc * 128:(mc + 1) * 128],
                                    seg_o[seg][kc][:, :],
                                    start=(kc == 0), stop=(kc == NC_D - 1))
                            b_ap = ap_vec("BO", l * 4 + mc) if flags["bo"] else None
                            nc.vector.scalar_tensor_tensor(
                                xf[mc], ps, b_ap if b_ap is not None else 0.0,
                                xf[mc], ALU.add, ALU.add)
                        return f
                    return [chunk(mc) for mc in range(NC_D)]

                def wo_stats(seg):
                    xf = [xF[c][seg][:, :] for c in range(NC_D)]
                    sqts = [mk_sq(xf[mc]) for mc in range(NC_D)]
                    return ln_stats(xf, D, sqts=sqts)

                # phase A: qkv + attention; attention pair slots soak the
                # next-parity qkv chunks (attn 0/1) and the wo chunks of
                # completed segs (attn 2/3), keeping the PE fed through the
                # exp/broadcast serial chains. The deferred ln2 chain runs
                # under qkv PE work and must precede attn(s0) so its psmall
                # stats banks free up before attention recycles them.
                emit_qkv(0, l, wb)
                emit_qkv(1, l, wb)
                flush_ln2(l - 1)
                e2 = qkv_chunks(2, l, wb)
                emit_attn(0, inject=e2[0] + e2[1])
                e3 = qkv_chunks(3, l, wb)
                emit_attn(1, inject=e3[0] + e3[1])
                def ln1_bcapply(seg, pre):
                    xf = [xF[c][seg][:, :] for c in range(NC_D)]
                    bc = ln_chain_bc(pre)
                    ln_apply(bc, xf,
                             (lambda c: ap_vec("G1", l * 4 + c)) if flags["g1"] else None,
                             (lambda c: ap_vec("BE1", l * 4 + c)) if flags["be1"] else None,
                             bf_out=[xT[c][seg][:, :] for c in range(NC_D)])

                emit_attn(2, inject=wo_chunks(0) + wo_chunks(1))
                # seg0's LN1 chain: stats+pre before attn(3), bc+apply
                # injected into it, so ffn(0) can start right at attn(3) end.
                b_stats = {0: wo_stats(0)}
                p_ln1 = {0: ln_chain_pre(b_stats[0], D)}
                emit_attn(3, inject=wo_chunks(2) +
                          [lambda: ln1_bcapply(0, p_ln1[0])])
                for f in wo_chunks(3):
                    f()

                def ffn_block(seg, defer_ln2=False):
                    xf = [xF[c][seg][:, :] for c in range(NC_D)]
                    xs = [xT[c][seg][:, :] for c in range(NC_D)]
                    for mc in range(NC_FF):
                        ps = P["ppro"].tile([128, TT], F32, tag="pp")
                        for kc in range(NC_D):
                            nc.tensor.matmul(
                                ps, w1_t[kc][:, mc * 128:(mc + 1) * 128], xs[kc],
                                start=(kc == 0), stop=(kc == NC_D - 1))
                        nc.scalar.activation(
                            hT[mc][:, :], ps, AF.Gelu,
                            bias=ap_vec("B1", l * 16 + mc) if flags["b1"] else 0.0,
                            scale=1.0)
                    # W2 in two waves of 2 output chunks (2 live accumulators)
                    sqts = []
                    for wave in range(2):
                        wps = [P["pacc"].tile([128, TT], F32, tag="pa",
                                              name=f"w2ps{m}") for m in range(2)]
                        for kc in range(NC_FF):
                            for m in range(2):
                                nc.tensor.matmul(
                                    wps[m],
                                    w2_t[kc][:, (wave * 2 + m) * 128:(wave * 2 + m + 1) * 128],
                                    hT[kc][:, :],
                                    start=(kc == 0), stop=(kc == NC_FF - 1))
                        for m in range(2):
                            mcD = wave * 2 + m
                            b_ap = ap_vec("B2", l * 4 + mcD) if flags["b2"] else None
                            nc.vector.scalar_tensor_tensor(
                                xf[mcD], wps[m], b_ap if b_ap is not None else 0.0,
                                xf[mcD], ALU.add, ALU.add)
                            sqts.append(mk_sq(xf[mcD]))
                    st = ln_stats(xf, D, sqts=sqts)
                    if defer_ln2:
                        pending_ln2.append((seg, st))
                        return
                    bc = ln_chain(st, D)
                    ln_apply(bc, xf,
                             (lambda c: ap_vec("G2", l * 4 + c)) if flags["g2"] else None,
                             (lambda c: ap_vec("BE2", l * 4 + c)) if flags["be2"] else None,
                             bf_out=xs)

                # seg0's ln1 was handled inside attn(3); remaining ln1
                # chains split pre/bc and pipeline through the ffn blocks.
                b_stats[1] = wo_stats(1)
                b_stats[2] = wo_stats(2)
                p_ln1[1] = ln_chain_pre(b_stats[1], D)
                b_stats[3] = wo_stats(3)
                ln1_bcapply(1, p_ln1[1])
                p_ln1[2] = ln_chain_pre(b_stats[2], D)
                if not (emit_head and l == n_layers - 1):
                    ffn_block(0)
                    ln1_bcapply(2, p_ln1[2])
                    p_ln1[3] = ln_chain_pre(b_stats[3], D)
                    ffn_block(1)
                    ln1_bcapply(3, p_ln1[3])
                    ffn_block(2)
                    ffn_block(3, defer_ln2=True)
                else:
                    # last layer: wavefront the head stages through the FFN
                    # blocks. Every chain is split pre (DVE/ACT) / bc (PE)
                    # with PE-dense work emitted between them, and chains
                    # strictly alternate pre->bc so the bufs=1 mini slots
                    # never stall a pre on an unissued bc. Dummy keep-warm
                    # matmuls (kw) hold the HAM clock at 2.4GHz through the
                    # chain-latency-bound tail.
                    kw_ps = P["ppro"].tile([128, TT], F32, tag="pp", name="kwps")

                    def kw(n=3):
                        for _ in range(n):
                            nc.tensor.matmul(kw_ps, identb, xT[3][0][:, :],
                                             start=True, stop=True)

                    # segs {0,1} head pipelines entirely under ffn(3); segs
                    # {2,3} batch pairwise after it (m/yr bufs=2 lets two
                    # chain pres run back-to-back without waiting the first
                    # chain's broadcast matmuls).
                    s1, s2 = {}, {}
                    p1, p2, b1, b2k = {}, {}, {}, {}
                    ffn_block(0)
                    ln1_bcapply(2, p_ln1[2])
                    p_ln1[3] = ln_chain_pre(b_stats[3], D)
                    ffn_block(1)
                    ln1_bcapply(3, p_ln1[3])
                    ffn_block(2)
                    s1[0] = hp1(0); s1[1] = hp1(1)
                    p1[0] = ln_chain_pre(s1[0], D // 2)
                    p1[1] = ln_chain_pre(s1[1], D // 2)
                    ffn_block(3, defer_ln2=True)
                    (dseg, dst), = pending_ln2; pending_ln2.clear()
                    b1[0] = ln_chain_bc(p1[0]); apply1(0, b1[0])
                    b1[1] = ln_chain_bc(p1[1]); apply1(1, b1[1])
                    ln2p = ln_chain_pre(dst, D)
                    s2[0] = hp2(0); s2[1] = hp2(1)
                    ln2b = ln_chain_bc(ln2p)
                    ln_apply(ln2b, [xF[c][dseg][:, :] for c in range(NC_D)],
                             (lambda c: ap_vec("G2", l * 4 + c)) if flags["g2"] else None,
                             (lambda c: ap_vec("BE2", l * 4 + c)) if flags["be2"] else None,
                             bf_out=[xT[c][dseg][:, :] for c in range(NC_D)])
                    p2[0] = ln_chain_pre(s2[0], D // 4)
                    p2[1] = ln_chain_pre(s2[1], D // 4)
                    s1[2] = hp1(2)
                    b2k[0] = ln_chain_bc(p2[0]); apply2(0, b2k[0])
                    b2k[1] = ln_chain_bc(p2[1]); apply2(1, b2k[1])
                    s1[3] = hp1(3)
                    p1[2] = ln_chain_pre(s1[2], D // 2)
                    p1[3] = ln_chain_pre(s1[3], D // 2)
                    hp3(0); hp3(1)
                    b1[2] = ln_chain_bc(p1[2]); apply1(2, b1[2])
                    b1[3] = ln_chain_bc(p1[3]); apply1(3, b1[3])
                    kw(4)
                    s2[2] = hp2(2); s2[3] = hp2(3)
                    p2[2] = ln_chain_pre(s2[2], D // 4)
                    p2[3] = ln_chain_pre(s2[3], D // 4)
                    kw(4)
                    b2k[2] = ln_chain_bc(p2[2]); apply2(2, b2k[2])
                    b2k[3] = ln_chain_bc(p2[3]); apply2(3, b2k[3])
                    kw(4)
                    hp3(2); hp3(3)

            if not emit_head:
                flush_ln2(n_layers - 1)

            if dbg_x:
                for c in range(NC_D):
                    for s in range(NSEG):
                        nc.sync.dma_start(
                            out=xdbg_d[c * 128:(c + 1) * 128, s * TT:(s + 1) * TT],
                            in_=xT[c][s][:, :])

    nc.compile()
    return nc, extra


# ======================================================================
# Self-contained kernel entry point: takes FULL inputs, shards batch over
# 8 NeuronCores (data-parallel), runs the Bass kernel, gathers output.
# ======================================================================
from concourse.bass_utils import run_bass_kernel_spmd

N_CORES = 8


def make_in_maps(inputs, extra):
    base = dict(extra)
    obs = np.asarray(inputs["observations"], np.float32)
    n_b = obs.shape[0]
    per = n_b // N_CORES
    in_maps = []
    for c in range(N_CORES):
        m = dict(base)
        m["observations"] = np.ascontiguousarray(
            obs[c * per:(c + 1) * per]).astype(BF_NP)
        in_maps.append(m)
    return in_maps, per


def kernel(**inputs):
    inputs = {k: np.asarray(v) for k, v in inputs.items()}
    nc, extra = build(inputs, n_layers=8, emit_head=True, dbg_x=False)
    in_maps, per = make_in_maps(inputs, extra)

    last_err = None
    for attempt in range(4):
        try:
            res = run_bass_kernel_spmd(nc, in_maps, core_ids=list(range(N_CORES)),
                                       trace=False)
            outs = [res.results[c]["OUT"].reshape(per, S, ACT_DIM)
                    for c in range(N_CORES)]
            return np.concatenate(outs, axis=0)
        except Exception as e:  # transient NRT_EXEC_UNIT_UNRECOVERABLE etc.
            last_err = e
            import time as _time
            _time.sleep(3.0 * (attempt + 1))
    raise last_err



# revision 52
# speedup vs baseline: 1.0350x; 1.0350x over previous
"""Transformer policy kernel for TRN2 (Bass/Tile), v3: bf16 matmuls +
fp32 residual + phase-batched software pipeline.
Verified: 2387458 ns HW, rel err 5.35e-3 (baseline 2986054 ns).

Per core (data-parallel over batch): BC=8 batches x S=256 -> T=2048 tokens.
D=512 (4 chunks), H=8 heads (HD=64), FF=2048 (16 chunks), L=8 layers.

Design (what actually survived hardware):
- Matmul operands bf16 (host-cast weights): enables Fast Weight Load
  (fp32r gets none -- it was inflating 512-col matmuls 423ns vs 215ns) and
  halves SBUF/DMA. PSUM accumulation fp32.
- Residual stream xF kept in fp32r (storing it bf16 costs ~3e-2 rel err
  over 8 layers); ln_apply writes a bf16 mirror xT via ACT as the matmul
  operand. LN stats read xF directly as fp32r (1 cyc/row at >=256 cols).
- Tableless LN rstd: Quake-seed + 1 Newton step, split DVE (int-ALU seed,
  muls; final mul writes a fresh fp32r tile -- fp32r matmul operands need a
  rounding producer) / ACT (Copy/Square live in every table). ACT table
  loads: 212 -> 18. M/R broadcast via 1-row PE matmuls (gpsimd broadcast
  straggles to 4us+).
- Per layer: [A: qkv+attn x4 segs, one-seg qkv lookahead so ACT-queued q/k
  copies never gate scores; deferred prev-layer LN2(s3) chain flushed under
  qkv(s0) PE work] then [wo+LN1 / FFN+LN2 interleaved per the emission
  order below] -- every serial LN chain hides under another segment's PE
  block. qkvo weights double-buffered; w1/w2 prefetched at layer start.
- Attention: scores auto row-packed (head pair at partition 0/64); AV at
  base partition 0 only (PSUM col-group 3 / output partitions 96:127 is a
  buggy PE quadrant -- explicit col-tiling there corrupts results); odd
  head reaches seg_o via partition-shift SBUF DMA. Denominators via ones
  matmuls; reciprocal_approx_fast + gpsimd broadcast.

Known rejected/failed directions (measured):
- fp8 DoubleRow W2: 2173915 ns but rel err 2.48e-2 > 2e-2 gate (fp8's ~3%
  per-element noise passes through dot products undiminished).
- Deferring LN2 for segs 2+3 (not just 3): NRT_EXEC_UNIT_UNRECOVERABLE.
- Ones-column-in-V denominator fold: DVE/PSUM reads need 32-aligned
  partition bases; the [1|64f] and [64f|1] layouts both dead-end.
Remaining levers: ~227us PE idle in 128-700ns dependency bubbles, ~120us
cold-clock (HAM), ~280us LDWEIGHTS/dispatch overhead vs the 1768us
column-streaming floor (currently at 74% of that roofline).
"""
import math
import contextlib
import numpy as np
import ml_dtypes

import concourse.bass as bass
import concourse.bacc as bacc
import concourse.tile as tile
from concourse import mybir

F32 = mybir.dt.float32
BF16 = mybir.dt.bfloat16
AF = mybir.ActivationFunctionType
ALU = mybir.AluOpType

BF_NP = ml_dtypes.bfloat16

BC = 8
S = 256
T = BC * S
OBS = 96
ACT_DIM = 29
D = 512
H = 8
HD = 64
FF = 2048
NC_D = D // 128
NC_FF = FF // 128
TT = 512
NSEG = T // TT
EPS = 1e-5
L_MAX = 8


def _nz(a):
    return a is not None and bool(np.any(np.asarray(a) != 0))


def _ng(a):
    return a is not None and bool(np.any(np.asarray(a) != 1))


def build(inputs, n_layers=8, emit_head=True, dbg_x=False):
    """inputs: dict of full np arrays (reference naming). Returns (nc, extra_in_map)."""
    nc = bacc.Bacc("TRN2", target_bir_lowering=False, debug=False)

    flags = dict(
        bin_=_nz(inputs["b_in"]), gin=_ng(inputs["g_in"]), bein=_nz(inputs["be_in"]),
        bq=_nz(inputs["bq"]), bk=_nz(inputs["bk"]), bv=_nz(inputs["bv"]), bo=_nz(inputs["bo"]),
        g1=_ng(inputs["g1"]), be1=_nz(inputs["be1"]), b1=_nz(inputs["b1"]), b2=_nz(inputs["b2"]),
        g2=_ng(inputs["g2"]), be2=_nz(inputs["be2"]),
        bp1=_nz(inputs["bp1"]), gp1=_ng(inputs["gp1"]), bep1=_nz(inputs["bep1"]),
        bp2=_nz(inputs["bp2"]), gp2=_ng(inputs["gp2"]), bep2=_nz(inputs["bep2"]),
        bp3=_nz(inputs["bp3"]), asc=_ng(inputs["action_scale"]), abi=_nz(inputs["action_bias"]),
    )

    def din(name, shape, dt=BF16):
        return nc.dram_tensor(name, shape, dt, kind="ExternalInput").ap()

    F32R = mybir.dt.float32r
    obs_d = din("observations", (BC, S, OBS))
    win_d = din("W_in", (OBS, D))
    wq_d = din("Wq", (L_MAX, D, D)); wk_d = din("Wk", (L_MAX, D, D))
    wv_d = din("Wv", (L_MAX, D, D)); wo_d = din("Wo", (L_MAX, D, D))
    w1_d = din("W1", (L_MAX, D, FF)); w2_d = din("W2", (L_MAX, FF, D))
    wp1_d = din("Wp1", (D, D // 2)); wp2_d = din("Wp2", (D // 2, D // 4))
    wp3_d = din("Wp3", (D // 4, ACT_DIM))
    identb_d = din("IDENTB", (128, 128))
    identf_d = din("IDENTF", (128, 128), F32)
    ones_d = din("ONES", (128, 8))
    onesf_d = din("ONESF", (128, 128), F32R)
    pet_d = din("PET", (D, S))
    out_d = nc.dram_tensor("OUT", (T, ACT_DIM), F32, kind="ExternalOutput").ap()
    if dbg_x:
        xdbg_d = nc.dram_tensor("XDBG", (D, T), BF16, kind="ExternalOutput").ap()

    extra = {
        "IDENTB": np.eye(128, dtype=BF_NP),
        "IDENTF": np.eye(128, dtype=np.float32),
        "ONES": np.ones((128, 8), BF_NP),
        "ONESF": np.ones((128, 128), np.float32),
    }
    pos = np.arange(S, dtype=np.float32)[:, None]
    div = np.exp(np.arange(0, D, 2, dtype=np.float32) * (-math.log(10000.0) / D))
    pe = np.zeros((S, D), dtype=np.float32)
    pe[:, 0::2] = np.sin(pos * div)
    pe[:, 1::2] = np.cos(pos * div)
    extra["PET"] = np.ascontiguousarray(pe.T).astype(BF_NP)
    for k in ["W_in", "Wq", "Wk", "Wv", "Wo", "W1", "W2", "Wp1", "Wp2", "Wp3"]:
        extra[k] = np.ascontiguousarray(np.asarray(inputs[k], np.float32)).astype(BF_NP)

    # per-feature bias/gain vectors (feature-major [128, n]) -- only emitted
    # when the corresponding values are nontrivial (not for this problem).
    def vec_tensor(name, arr):
        a = np.asarray(arr, np.float32).reshape(-1)
        n = a.size // 128
        extra[name] = np.ascontiguousarray(a.reshape(n, 128).T)
        return din(name, (128, n), F32)

    dv = {}
    for key, nm in [("bq", "BQ"), ("bk", "BK"), ("bo", "BO"), ("b1", "B1"), ("b2", "B2"),
                    ("b_in", "BIN"), ("g_in", "GIN"), ("be_in", "BEIN"),
                    ("g1", "G1"), ("be1", "BE1"), ("g2", "G2"), ("be2", "BE2"),
                    ("bp1", "BP1"), ("gp1", "GP1"), ("bep1", "BEP1"),
                    ("bp2", "BP2"), ("gp2", "GP2"), ("bep2", "BEP2")]:
        fkey = {"b_in": "bin_", "g_in": "gin", "be_in": "bein"}.get(key, key)
        if flags[fkey]:
            dv[nm] = vec_tensor(nm + "v", inputs[key])
    if flags["bv"]:
        extra["BVr"] = np.asarray(inputs["bv"], np.float32).astype(BF_NP).reshape(L_MAX, D)
        dv["BV"] = din("BVr", (L_MAX, D))

    def vec29(name, arr):
        a = np.zeros((128, 1), np.float32)
        a[:ACT_DIM, 0] = np.asarray(arr, np.float32).reshape(-1)
        extra[name] = a
        return din(name, (128, 1), F32)
    if flags["bp3"]:
        dv["BP3"] = vec29("BP3v", inputs["bp3"])
    if flags["asc"]:
        dv["ASC"] = vec29("ASCv", inputs["action_scale"])
    if flags["abi"]:
        dv["ABI"] = vec29("ABIv", inputs["action_bias"])

    scale = 1.0 / math.sqrt(HD)

    with tile.TileContext(nc) as tc:
        with contextlib.ExitStack() as ctx:
            P = {}
            P["persist"] = ctx.enter_context(tc.tile_pool(name="persist", bufs=1))
            P["wpool"] = ctx.enter_context(tc.tile_pool(name="wpool", bufs=1))
            P["xpool"] = ctx.enter_context(tc.tile_pool(name="xpool", bufs=1))
            P["segt"] = ctx.enter_context(tc.tile_pool(name="segt", bufs=1))
            P["hpool"] = ctx.enter_context(tc.tile_pool(name="hpool", bufs=1))
            P["exps"] = ctx.enter_context(tc.tile_pool(name="exps", bufs=2))
            P["sq"] = ctx.enter_context(tc.tile_pool(name="sq", bufs=2))
            P["scratch"] = ctx.enter_context(tc.tile_pool(name="scratch", bufs=2))
            P["mini"] = ctx.enter_context(tc.tile_pool(name="mini", bufs=1))
            P["rbs"] = ctx.enter_context(tc.tile_pool(name="rbs", bufs=2))
            P["rec"] = ctx.enter_context(tc.tile_pool(name="rec", bufs=1))
            P["ppro"] = ctx.enter_context(tc.tile_pool(name="ppro", bufs=2, space="PSUM"))
            P["pacc"] = ctx.enter_context(tc.tile_pool(name="pacc", bufs=2, space="PSUM"))
            P["psmall"] = ctx.enter_context(tc.tile_pool(name="psmall", bufs=4, space="PSUM"))

            # ---------------- constants ----------------
            identb = P["persist"].tile([128, 128], BF16, tag="identb")
            nc.sync.dma_start(out=identb, in_=identb_d[:, :])
            identf = P["persist"].tile([128, 128], F32, tag="identf")
            nc.sync.dma_start(out=identf, in_=identf_d[:, :])
            ones = P["persist"].tile([128, 8], BF16, tag="ones")
            nc.sync.dma_start(out=ones, in_=ones_d[:, :])
            ones_col = ones[:, 0:1]
            F32R = mybir.dt.float32r
            onesf = P["persist"].tile([128, 128], F32R, tag="onesf")
            nc.sync.dma_start(out=onesf, in_=onesf_d[:, :])
            onesf_col = onesf[:, 0:1]
            onesf_row = onesf[0:1, :]

            peT = P["persist"].tile([128, NC_D * S], BF16, tag="peT")
            nc.sync.dma_start(out=peT.rearrange("p (c s) -> p c s", s=S),
                              in_=pet_d.rearrange("(c p) s -> p c s", p=128))

            vt = {}
            for nm, d in dv.items():
                if nm == "BV":
                    t = P["persist"].tile([1, L_MAX * D], BF16, tag="c_BV")
                    for l in range(L_MAX):
                        nc.sync.dma_start(out=t[:, l * D:(l + 1) * D], in_=d[l:l + 1, :])
                else:
                    t = P["persist"].tile([128, d.shape[1]], F32, tag=f"c_{nm}")
                    nc.sync.dma_start(out=t, in_=d[:, :])
                vt[nm] = t

            def ap_vec(nm, idx):
                t = vt.get(nm)
                return t[:, idx:idx + 1] if t is not None else None

            # ---------------- big tiles ----------------
            # xF: fp32 residual stream (rounding the residual to bf16 costs
            # ~3e-2 rel err over 8 layers); xT: bf16 mirror fed to matmuls.
            xF = [[P["xpool"].tile([128, TT], F32R, tag=f"xF{c}_{s}", name=f"xF{c}_{s}")
                   for s in range(NSEG)] for c in range(NC_D)]
            xT = [[P["xpool"].tile([128, TT], BF16, tag=f"xT{c}_{s}", name=f"xT{c}_{s}")
                   for s in range(NSEG)] for c in range(NC_D)]
            # double-buffered q/k/v (indexed seg%2), per-seg o
            seg_q = [[P["segt"].tile([128, TT], BF16, tag=f"sq{c}_{b}", name=f"sq{c}_{b}")
                      for c in range(NC_D)] for b in range(2)]
            seg_k = [[P["segt"].tile([128, TT], BF16, tag=f"sk{c}_{b}", name=f"sk{c}_{b}")
                      for c in range(NC_D)] for b in range(2)]
            # seg_v carries 8 heads x (64 feat + 1 ones col): the ones col
            # folds the softmax denominator into the AV matmul (row 64 of
            # the [65, S] AV output = sum of exp), killing the aux ones
            # matmuls (4 per pair, ~112us of PE across the kernel).
            seg_v = [[P["segt"].tile([128, H * (HD + 1)], BF16, tag=f"sv{c}_{b}",
                                     name=f"sv{c}_{b}")
                      for c in range(NC_D)] for b in range(2)]
            seg_o = [[P["segt"].tile([128, TT], BF16, tag=f"so{c}_{s}", name=f"so{c}_{s}")
                      for c in range(NC_D)] for s in range(NSEG)]
            hT = [P["hpool"].tile([128, TT], BF16, tag=f"hT{m}", name=f"hT{m}")
                  for m in range(NC_FF)]

            # weights: qkvo+w1 double-buffered, w2 single (prefetched per layer)
            # each matrix is ONE [128, NC*D] tile (chunk-major cols) so a
            # whole matrix loads in a single DMA: the sync queue costs
            # ~610ns per dma_start, so 36 small DMAs/layer was ~20us of
            # issue serialization.
            wq_b = [P["wpool"].tile([128, NC_D * D], BF16, tag=f"wq_{b}", name=f"wq_{b}")
                    for b in range(2)]
            wk_b = [P["wpool"].tile([128, NC_D * D], BF16, tag=f"wk_{b}", name=f"wk_{b}")
                    for b in range(2)]
            wv_b = [P["wpool"].tile([128, NC_D * D], BF16, tag=f"wv_{b}", name=f"wv_{b}")
                    for b in range(2)]
            wo_b = [P["wpool"].tile([128, NC_D * D], BF16, tag=f"wo_{b}", name=f"wo_{b}")
                    for b in range(2)]
            wq_t = [[wq_b[b][:, k * D:(k + 1) * D] for k in range(NC_D)] for b in range(2)]
            wk_t = [[wk_b[b][:, k * D:(k + 1) * D] for k in range(NC_D)] for b in range(2)]
            wv_t = [[wv_b[b][:, k * D:(k + 1) * D] for k in range(NC_D)] for b in range(2)]
            wo_t = [[wo_b[b][:, k * D:(k + 1) * D] for k in range(NC_D)] for b in range(2)]
            w1_bt = P["wpool"].tile([128, NC_D * FF], BF16, tag="w1", name="w1t")
            w1_t = [w1_bt[:, k * FF:(k + 1) * FF] for k in range(NC_D)]
            w2_bt = P["wpool"].tile([128, NC_FF * D], BF16, tag="w2", name="w2t")
            w2_t = [w2_bt[:, k * D:(k + 1) * D] for k in range(NC_FF)]
            # head/input weights: win aliases w2_t[0] (w2 of layer 0 is
            # loaded only after the input-stage projections are emitted)
            win_t = w2_t[0][:, :]
            wp1_t = [P["wpool"].tile([128, D // 2], BF16, tag=f"wp1{k}", name=f"wp1{k}")
                     for k in range(NC_D)]
            wp2_t = [P["wpool"].tile([128, D // 4], BF16, tag=f"wp2{k}", name=f"wp2{k}")
                     for k in range(2)]
            wp3_t = P["wpool"].tile([128, 32], BF16, tag="wp3")

            def load_layer_weights(lx, b):
                for dst, srcd in [(wq_b[b], wq_d), (wk_b[b], wk_d),
                                   (wv_b[b], wv_d), (wo_b[b], wo_d)]:
                    nc.sync.dma_start(
                        out=dst.rearrange("p (k j) -> p k j", j=D),
                        in_=srcd[lx].rearrange("(k p) j -> p k j", p=128))

            def load_w12(lx):
                nc.sync.dma_start(out=w1_bt.rearrange("p (k j) -> p k j", j=FF),
                                  in_=w1_d[lx].rearrange("(k p) j -> p k j", p=128))
                nc.sync.dma_start(out=w2_bt.rearrange("p (k j) -> p k j", j=D),
                                  in_=w2_d[lx].rearrange("(k p) j -> p k j", p=128))

            # ---------------- layernorm (split into stats / chain / apply) ----
            def mk_sq(chunk_ap):
                sqt = P["sq"].tile([128, TT], F32R, tag="sqt")
                nc.scalar.square(sqt, chunk_ap)
                return sqt

            def ln_stats(chunk_aps, nfeat, sqts=None):
                """stats over fp32r chunks: squares on ACT, sums via ones
                matmuls (fp32r, 1 cyc/row at >=256 cols). All sums matmuls
                go first so the PE isn't gated on ACT square latency."""
                nch = len(chunk_aps)
                sums = P["psmall"].tile([1, TT], F32, tag="sm", name="sums")
                sumsq = P["psmall"].tile([1, TT], F32, tag="sm", name="sumsq")
                if sqts is None:
                    sqts = [mk_sq(chunk_aps[c]) for c in range(nch)]
                for c in range(nch):
                    nc.tensor.matmul(sums, onesf_col, chunk_aps[c],
                                     start=(c == 0), stop=(c == nch - 1))
                for c in range(nch):
                    nc.tensor.matmul(sumsq, onesf_col, sqts[c],
                                     start=(c == 0), stop=(c == nch - 1))
                return sums, sumsq

            def ln_chain_pre(stats, nfeat, n_newton=1):
                """mean + rstd for a token-column LN, tableless: rstd via
                Quake-seed + Newton, split across ACT (Copy/Square, in every
                table) and DVE (muls + int seed). Returns (m, yr) row
                vectors; ln_chain_bc broadcasts them on the PE."""
                sums, sumsq = stats
                I32 = mybir.dt.int32
                m = P["mini"].tile([1, TT], F32R, tag="m", bufs=2)
                nc.scalar.mul(m, sums, 1.0 / nfeat)
                e2 = P["mini"].tile([1, TT], F32, tag="e2")
                nc.scalar.activation(e2, sumsq, AF.Copy, bias=float(EPS),
                                     scale=1.0 / nfeat)
                msq = P["mini"].tile([1, TT], F32, tag="msq")
                nc.scalar.square(msq, m)
                nc.vector.tensor_sub(e2, e2, msq)  # e2 = var + eps
                y = P["mini"].tile([1, TT], F32, tag="y")
                nc.vector.tensor_scalar(out=y.bitcast(I32), in0=e2.bitcast(I32),
                                        scalar1=1, scalar2=None,
                                        op0=ALU.logical_shift_right)
                nc.vector.tensor_scalar(out=y.bitcast(I32), in0=y.bitcast(I32),
                                        scalar1=0x5F3759DF, scalar2=-1,
                                        op0=ALU.subtract, op1=ALU.mult)
                # last Newton step writes a fresh fp32r tile: fp32r matmul
                # operands must come from a producer that rounds to fp32r,
                # which the int-ALU seed ops above don't.
                yr = P["mini"].tile([1, TT], F32R, tag="yr", bufs=2)
                for it in range(n_newton):
                    a = P["mini"].tile([1, TT], F32, tag="nta")
                    nc.vector.tensor_mul(a, y, y)
                    nc.vector.tensor_mul(a, a, e2)
                    nc.scalar.activation(a, a, AF.Copy, bias=1.5, scale=-0.5)
                    nc.vector.tensor_mul(yr if it == n_newton - 1 else y, y, a)
                return m, yr

            def ln_chain_bc(pre):
                # broadcast M/R across partitions on the PE (gpsimd broadcast
                # latency straggles to 4us+ and stalled the FFN start). The
                # ones stationary row is picked at the moving operand's base
                # partition (matmul requires matching bases).
                m, yr = pre
                bm, br = m.base_partition(), yr.base_partition()
                M = P["pacc"].tile([128, TT], F32, tag="pa", name="Mb")
                nc.tensor.matmul(M, onesf[bm:bm + 1, :], m, start=True, stop=True)
                R = P["pacc"].tile([128, TT], F32, tag="pa", name="Rb")
                nc.tensor.matmul(R, onesf[br:br + 1, :], yr, start=True, stop=True)
                return M, R

            def ln_chain(stats, nfeat, n_newton=1):
                return ln_chain_bc(ln_chain_pre(stats, nfeat, n_newton))

            def ln_apply(bc, chunk_aps, g_fn=None, b_fn=None, gelu=False,
                         bf_out=None):
                """normalize fp32r chunks in place; optionally gelu; optionally
                write a bf16 mirror (the matmul operand) via ACT."""
                M, R = bc
                for c in range(len(chunk_aps)):
                    xc = chunk_aps[c]
                    g_ap = g_fn(c) if g_fn is not None else None
                    b_ap = b_fn(c) if b_fn is not None else None
                    nc.vector.tensor_sub(xc, xc, M)
                    if g_ap is not None:
                        nc.vector.scalar_tensor_tensor(xc, xc, g_ap, R,
                                                       ALU.mult, ALU.mult)
                    else:
                        nc.vector.tensor_mul(xc, xc, R)
                    if gelu:
                        nc.scalar.activation(xc, xc, AF.Gelu,
                                             bias=b_ap if b_ap is not None else 0.0,
                                             scale=1.0)
                    elif b_ap is not None:
                        nc.scalar.activation(xc, xc, AF.Identity, bias=b_ap,
                                             scale=1.0)
                    if bf_out is not None:
                        nc.scalar.copy(bf_out[c], xc)

            # ---------------- projections ----------------
            def proj_fm(w_tiles, in_aps, out_aps, bias_fn=None, kpart=128,
                        epi="act"):
                """feature-major projection: out[mc] = W.T @ in (+bias).
                Epilogue copy on ACT by default (DVE is the busier engine)."""
                n_out = len(out_aps)
                n_in = len(in_aps)
                for mc in range(n_out):
                    ps = P["ppro"].tile([128, TT], F32, tag="pp")
                    for kc in range(n_in):
                        nc.tensor.matmul(
                            ps, w_tiles[kc][0:kpart, mc * 128:(mc + 1) * 128],
                            in_aps[kc][0:kpart, :],
                            start=(kc == 0), stop=(kc == n_in - 1))
                    b_ap = bias_fn(mc) if bias_fn is not None else None
                    if b_ap is None and epi == "dve":
                        nc.vector.tensor_copy(out_aps[mc], ps)
                    elif b_ap is None:
                        nc.scalar.copy(out_aps[mc], ps)
                    else:
                        nc.scalar.activation(out_aps[mc], ps, AF.Identity,
                                             bias=b_ap, scale=1.0)

            # ---------------- attention ----------------
            def qkv_chunks(seg, l, wb):
                """return the q/k/v projection of one seg as 12 single-psum
                chunk callables: [q0,k0,q1,k1,q2,k2] (safe to inject into
                the PREVIOUS seg-parity attention at pair slots 2.. with
                c-major pair order) and [q3,k3,v0..v3] (safe once that
                attention is fully emitted)."""
                db = seg % 2
                xs = [xT[c][seg][:, :] for c in range(NC_D)]

                def q_chunk(mc):
                    def f():
                        ps = P["ppro"].tile([128, TT], F32, tag="pp")
                        for kc in range(NC_D):
                            nc.tensor.matmul(
                                ps, wq_t[wb][kc][:, mc * 128:(mc + 1) * 128],
                                xs[kc], start=(kc == 0), stop=(kc == NC_D - 1))
                        b_ap = ap_vec("BQ", l * 4 + mc) if flags["bq"] else None
                        if b_ap is None:
                            nc.scalar.copy(seg_q[db][mc][:, :], ps)
                        else:
                            nc.scalar.activation(seg_q[db][mc][:, :], ps,
                                                 AF.Identity, bias=b_ap, scale=1.0)
                    return f

                def k_chunk(mc):
                    def f():
                        ps = P["ppro"].tile([128, TT], F32, tag="pp")
                        for kc in range(NC_D):
                            nc.tensor.matmul(
                                ps, wk_t[wb][kc][:, mc * 128:(mc + 1) * 128],
                                xs[kc], start=(kc == 0), stop=(kc == NC_D - 1))
                        b_ap = ap_vec("BK", l * 4 + mc) if flags["bk"] else None
                        if b_ap is None:
                            nc.vector.tensor_copy(seg_k[db][mc][:, :], ps)
                        else:
                            nc.scalar.activation(seg_k[db][mc][:, :], ps,
                                                 AF.Identity, bias=b_ap, scale=1.0)
                    return f

                def v_chunk(ts):
                    def f():
                        vp = P["ppro"].tile([128, D], F32, tag="pp")
                        for kc in range(NC_D):
                            nc.tensor.matmul(
                                vp, xT[kc][seg][:, ts * 128:(ts + 1) * 128],
                                wv_t[wb][kc],
                                start=(kc == 0), stop=(kc == NC_D - 1) and not flags["bv"])
                        if flags["bv"]:
                            nc.tensor.matmul(vp, ones[0:1, 0:128],
                                             vt["BV"][:, l * D:(l + 1) * D],
                                             start=False, stop=True)
                        nc.vector.tensor_copy(
                            seg_v[db][ts].rearrange("p (h g) -> p h g", g=HD + 1)[:, :, 0:HD],
                            vp.rearrange("p (h f) -> p h f", f=HD))
                    return f

                early = [q_chunk(0), k_chunk(0), q_chunk(1), k_chunk(1),
                         q_chunk(2), k_chunk(2)]
                late = [q_chunk(3), k_chunk(3)] + [v_chunk(ts) for ts in range(4)]
                return early, late

            def emit_qkv(seg, l, wb):
                early, late = qkv_chunks(seg, l, wb)
                for f in early + late:
                    f()

            def emit_attn(seg, inject=None):
                """softmax attention for one segment (2 batches x 4 head pairs).
                One-pair lookahead: pair p+1's score matmuls are emitted before
                pair p's AV matmuls so the PE never waits on Exp. Pair order is
                c-major so seg_q/k chunk c has no readers after pair 2c+1 --
                lets the next-parity qkv chunks inject early. `inject` is a
                list of callables emitting ready PE work, consumed one per
                pair slot to fill the exp-latency bubbles."""
                db = seg % 2
                pairs = [(b2, c) for c in range(NC_D) for b2 in range(2)]
                inject = list(inject) if inject else []
                st = {}

                def do_inject(n=1):
                    for _ in range(n):
                        if inject:
                            inject.pop(0)()

                def scores(p):
                    b2, c = pairs[p]
                    bcol = b2 * S
                    scps, ess = [], []
                    for hh in range(2):
                        roff = hh * HD
                        scp = P["psmall"].tile([128, 2 * S], F32, tag="sm",
                                               name=f"scp{hh}")
                        for kc in range(2):
                            nc.tensor.matmul(
                                scp[:, kc * S:(kc + 1) * S],
                                seg_k[db][c][roff:roff + HD,
                                             bcol + kc * 128: bcol + (kc + 1) * 128],
                                seg_q[db][c][roff:roff + HD, bcol:bcol + S],
                                start=True, stop=True)
                        scps.append(scp)
                    for hh in range(2):
                        esh = P["exps"].tile([128, 2 * S], BF16, tag=f"es{hh}",
                                             name=f"es{hh}")
                        nc.scalar.activation(esh, scps[hh], AF.Exp, bias=0.0,
                                             scale=scale)
                        ess.append(esh)
                    st[p] = ess

                def avpart(p):
                    # v1-proven AV layout: both heads' AV outputs at base
                    # partition 0 (PSUM col-group 3 -- output partitions
                    # 96:127 -- is a buggy PE quadrant, so no col-tiling);
                    # the odd head reaches seg_o partitions 64:128 via an
                    # SBUF->SBUF partition-shift DMA. aux+otp are 2 "sm"
                    # slots (not 3) so the 4-slot round-robin never makes
                    # pair p's AV matmuls wait on pair p+1's exp.
                    b2, c = pairs[p]
                    bcol = b2 * S
                    ess = st.pop(p)
                    otpb = P["psmall"].tile([HD + 1, 2 * S], F32, tag="sm", name="otpb")
                    otp = [otpb[0:HD, hh * S:(hh + 1) * S] for hh in range(2)]
                    for hh in range(2):
                        h = 2 * c + hh
                        for kc in range(2):
                            nc.tensor.matmul(
                                otpb[0:HD + 1, hh * S:(hh + 1) * S],
                                seg_v[db][b2 * 2 + kc][:, h * (HD + 1):(h + 1) * (HD + 1)],
                                ess[hh][:, kc * S:(kc + 1) * S],
                                start=(kc == 0), stop=(kc == 1))
                    # denom row 64 of the AV psum: DVE reads at partition
                    # base 64 of PSUM return garbage (HW quirk) and gpsimd
                    # can't read PSUM at all, so a sync DMA ferries the row
                    # to SBUF (ACT is the attention-phase bottleneck); recip
                    # runs 128-lane-wide after the broadcast. One wide mul
                    # divides both heads; one 3D-AP DMA scatters the halves
                    # into seg_o partitions 0:64 / 64:128.
                    rec = P["rec"].tile([1, 2 * S], F32, tag="rec")
                    nc.scalar.copy(rec, otpb[HD:HD + 1, :])
                    rbs = P["rbs"].tile([128, 2 * S], F32, tag="rbs")
                    nc.gpsimd.partition_broadcast(rbs, rec)
                    nc.vector.reciprocal_approx_fast(out=rbs, in_=rbs)
                    nc.vector.tensor_mul(
                        seg_o[seg][c][0:HD, bcol:bcol + S],
                        otp[0], rbs[0:HD, 0:S])
                    otmp = P["rbs"].tile([HD, S], BF16, tag="otmp")
                    nc.vector.tensor_mul(otmp, otp[1], rbs[0:HD, S:2 * S])
                    nc.sync.dma_start(out=seg_o[seg][c][HD:128, bcol:bcol + S],
                                      in_=otmp)

                scores(0)
                for p in range(1, 8):
                    scores(p)
                    avpart(p - 1)
                    # pair p's q/k chunk frees at pair 2c+1; injected chunk
                    # c's epilogue waits scores(2c+1), so keep c <= (p-2)//2
                    # to avoid parking the ACT/DVE queue on a long wait.
                    do_inject(1)
                avpart(7)
                do_inject(len(inject))

            # ---------------- input stage ----------------
            # ---- PE warm-up: ~4.5us of back-to-back matmuls releases the HAM
            # clock throttle (cold PE runs at 1.2 instead of 2.4 GHz) before
            # the thin-PE input stage and layer 0 begin.
            warm_ps = P["ppro"].tile([128, 128], F32, tag="pp", name="warmps")
            for _ in range(40):
                nc.tensor.matmul(warm_ps, identb, identb, start=True, stop=True)
            warm_out = P["scratch"].tile([128, 128], BF16, tag="warmo")
            nc.vector.tensor_copy(warm_out, warm_ps)

            # obs DMAs batched (1/seg) and issued before the weight loads so
            # the transposes aren't stuck behind ~16us of sync-queue issue.
            obs_flat = obs_d.rearrange("b s f -> (b s) f")
            ot_segs = []
            for seg in range(NSEG):
                # stage in dead seg_v tiles (free until layer-0 qkv)
                ot = seg_v[seg % 2][seg // 2][:, 0:4 * OBS]
                nc.sync.dma_start(
                    out=ot.rearrange("p (ts f) -> p ts f", f=OBS),
                    in_=obs_flat[seg * TT:(seg + 1) * TT]
                        .rearrange("(ts p) f -> p ts f", p=128))
                ot_segs.append(ot)
            nc.sync.dma_start(out=win_t[0:OBS, :], in_=win_d[:, :])
            in_stats = {}
            for seg in range(NSEG):
                obsT = seg_k[seg % 2][seg // 2]  # [96, 512] region staging
                for ts in range(4):
                    tp = P["psmall"].tile([OBS, 128], BF16, tag="sm", name="tpin")
                    nc.tensor.transpose(
                        tp, ot_segs[seg][:, ts * OBS:(ts + 1) * OBS], identb)
                    nc.vector.tensor_copy(obsT[0:OBS, ts * 128:(ts + 1) * 128], tp)
                xf = [xF[c][seg][:, :] for c in range(NC_D)]
                proj_fm([win_t], [obsT[:, :]], xf,
                        (lambda mc: ap_vec("BIN", mc)) if flags["bin_"] else None,
                        kpart=OBS)
                in_stats[seg] = ln_stats(xf, D)
            load_layer_weights(0, 0)
            load_w12(0)  # w2_t[0] aliases win; DMA waits the proj reads above
            # ones columns of seg_v (col h*65+64): written once, after the
            # obs staging reads; v-projection epilogues never touch them.
            for b in range(2):
                for ts in range(4):
                    nc.vector.tensor_copy(
                        seg_v[b][ts].rearrange("p (h g) -> p h g", g=HD + 1)[:, :, HD:HD + 1],
                        ones[:, 0:H].unsqueeze(2))
            in_bc = {}
            for seg in range(NSEG):
                in_bc[seg] = ln_chain(in_stats[seg], D)
            for seg in range(NSEG):
                xf = [xF[c][seg][:, :] for c in range(NC_D)]
                ln_apply(in_bc[seg], xf,
                         (lambda c: ap_vec("GIN", c)) if flags["gin"] else None,
                         (lambda c: ap_vec("BEIN", c)) if flags["bein"] else None,
                         gelu=True)
                for c in range(NC_D):
                    xc = xf[c]
                    nc.vector.tensor_add(
                        xc.rearrange("p (b s) -> p b s", s=S),
                        xc.rearrange("p (b s) -> p b s", s=S),
                        peT[:, c * S:(c + 1) * S].unsqueeze(1)
                           .broadcast_to([128, TT // S, S]))
                    nc.scalar.copy(xT[c][seg][:, :], xc)

            # ---------------- layers ----------------
            # ln2 chain of the last couple of segs is deferred into the next
            # layer's attention phase so it hides under qkv/attention PE work
            # instead of stalling the layer tail.
            pending_ln2 = []

            def flush_ln2(l_prev):
                for seg, st in pending_ln2:
                    xf = [xF[c][seg][:, :] for c in range(NC_D)]
                    bc = ln_chain(st, D)
                    ln_apply(bc, xf,
                             (lambda c: ap_vec("G2", l_prev * 4 + c)) if flags["g2"] else None,
                             (lambda c: ap_vec("BE2", l_prev * 4 + c)) if flags["be2"] else None,
                             bf_out=[xT[c][seg][:, :] for c in range(NC_D)])
                pending_ln2.clear()

            # ---------------- head helpers (per-seg, interleaved into the
            # last layer's FFN blocks so the LN chains hide under PE work
            # and the PE never goes cold at the kernel tail) --------------
            def hp1(seg):
                xs = [xT[c][seg][:, :] for c in range(NC_D)]
                y1f = [xF[mc][seg][:, :] for mc in range(2)]
                proj_fm(wp1_t, xs, y1f,
                        (lambda mc: ap_vec("BP1", mc)) if flags["bp1"] else None)
                return ln_stats(y1f, D // 2)

            def apply1(seg, bc):
                y1f = [xF[mc][seg][:, :] for mc in range(2)]
                y1b = [seg_q[seg % 2][mc][:, :] for mc in range(2)]
                ln_apply(bc, y1f,
                         (lambda c: ap_vec("GP1", c)) if flags["gp1"] else None,
                         (lambda c: ap_vec("BEP1", c)) if flags["bep1"] else None,
                         gelu=True, bf_out=y1b)

            def hp2(seg):
                y1b = [seg_q[seg % 2][mc][:, :] for mc in range(2)]
                y2f = [xF[2][seg][:, :]]
                proj_fm(wp2_t, y1b, y2f,
                        (lambda mc: ap_vec("BP2", 0)) if flags["bp2"] else None)
                return ln_stats(y2f, D // 4)

            def apply2(seg, bc):
                y2f = [xF[2][seg][:, :]]
                y2b = [seg_k[seg % 2][0][:, :]]
                ln_apply(bc, y2f,
                         (lambda c: ap_vec("GP2", 0)) if flags["gp2"] else None,
                         (lambda c: ap_vec("BEP2", 0)) if flags["bep2"] else None,
                         gelu=True, bf_out=y2b)

            def hp3(seg):
                y2b = seg_k[seg % 2][0][:, :]
                actp = P["psmall"].tile([ACT_DIM, TT], F32, tag="sm", name="actp")
                nc.tensor.matmul(actp, wp3_t[:, 0:ACT_DIM], y2b,
                                 start=True, stop=True)
                actT = P["mini"].tile([ACT_DIM, TT], F32, tag="actT")
                nc.scalar.activation(actT[0:ACT_DIM, :], actp, AF.Tanh,
                                     bias=vt["BP3"][0:ACT_DIM, 0:1] if flags["bp3"] else 0.0,
                                     scale=1.0)
                if flags["asc"] or flags["abi"]:
                    nc.scalar.activation(
                        actT[0:ACT_DIM, :], actT[0:ACT_DIM, :], AF.Identity,
                        bias=vt["ABI"][0:ACT_DIM, 0:1] if flags["abi"] else 0.0,
                        scale=vt["ASC"][0:ACT_DIM, 0:1] if flags["asc"] else 1.0)
                for ts in range(4):
                    tp = P["ppro"].tile([128, ACT_DIM], F32, tag="pp", name="tpo")
                    nc.tensor.transpose(tp, actT[0:ACT_DIM, ts * 128:(ts + 1) * 128],
                                        identf[0:ACT_DIM, 0:ACT_DIM])
                    ob = P["scratch"].tile([128, ACT_DIM], F32, tag="ob")
                    nc.vector.tensor_copy(ob, tp)
                    nc.sync.dma_start(
                        out=out_d[seg * TT + ts * 128: seg * TT + (ts + 1) * 128, :],
                        in_=ob)

            def load_head_weights():
                for k in range(NC_D):
                    nc.sync.dma_start(out=wp1_t[k], in_=wp1_d[k * 128:(k + 1) * 128, :])
                for k in range(2):
                    nc.sync.dma_start(out=wp2_t[k], in_=wp2_d[k * 128:(k + 1) * 128, :])
                nc.sync.dma_start(out=wp3_t[:, 0:ACT_DIM], in_=wp3_d[:, :])

            for l in range(n_layers):
                wb = l % 2
                if l + 1 < n_layers:
                    load_layer_weights(l + 1, 1 - wb)
                if l > 0:
                    load_w12(l)
                if emit_head and l == n_layers - 1:
                    load_head_weights()

                # phase B helpers (defined first; phase A injects wo chunks)
                def wo_chunks(seg):
                    xf = [xF[c][seg][:, :] for c in range(NC_D)]

                    def chunk(mc):
                        def f():
                            ps = P["ppro"].tile([128, TT], F32, tag="pp")
                            for kc in range(NC_D):
                                nc.tensor.matmul(
                                    ps, wo_t[wb][kc][:, mc * 128:(mc + 1) * 128],
                                    seg_o[seg][kc][:, :],
                                    start=(kc == 0), stop=(kc == NC_D - 1))
                            b_ap = ap_vec("BO", l * 4 + mc) if flags["bo"] else None
                            nc.vector.scalar_tensor_tensor(
                                xf[mc], ps, b_ap if b_ap is not None else 0.0,
                                xf[mc], ALU.add, ALU.add)
                        return f
                    return [chunk(mc) for mc in range(NC_D)]

                def wo_stats(seg):
                    xf = [xF[c][seg][:, :] for c in range(NC_D)]
                    sqts = [mk_sq(xf[mc]) for mc in range(NC_D)]
                    return ln_stats(xf, D, sqts=sqts)

                # phase A: qkv + attention; attention pair slots soak the
                # next-parity qkv chunks (attn 0/1) and the wo chunks of
                # completed segs (attn 2/3), keeping the PE fed through the
                # exp/broadcast serial chains. The deferred ln2 chain runs
                # under qkv PE work and must precede attn(s0) so its psmall
                # stats banks free up before attention recycles them.
                emit_qkv(0, l, wb)
                emit_qkv(1, l, wb)
                flush_ln2(l - 1)
                e2 = qkv_chunks(2, l, wb)
                emit_attn(0, inject=e2[0] + e2[1])
                e3 = qkv_chunks(3, l, wb)
                emit_attn(1, inject=e3[0] + e3[1])
                def ln1_bcapply(seg, pre):
                    xf = [xF[c][seg][:, :] for c in range(NC_D)]
                    bc = ln_chain_bc(pre)
                    ln_apply(bc, xf,
                             (lambda c: ap_vec("G1", l * 4 + c)) if flags["g1"] else None,
                             (lambda c: ap_vec("BE1", l * 4 + c)) if flags["be1"] else None,
                             bf_out=[xT[c][seg][:, :] for c in range(NC_D)])

                emit_attn(2, inject=wo_chunks(0) + wo_chunks(1))
                b_stats, p_ln1 = {}, {}
                emit_attn(3, inject=wo_chunks(2))
                for f in wo_chunks(3):
                    f()

                def ffn_block(seg, defer_ln2=False):
                    xf = [xF[c][seg][:, :] for c in range(NC_D)]
                    xs = [xT[c][seg][:, :] for c in range(NC_D)]
                    for mc in range(NC_FF):
                        ps = P["ppro"].tile([128, TT], F32, tag="pp")
                        for kc in range(NC_D):
                            nc.tensor.matmul(
                                ps, w1_t[kc][:, mc * 128:(mc + 1) * 128], xs[kc],
                                start=(kc == 0), stop=(kc == NC_D - 1))
                        nc.scalar.activation(
                            hT[mc][:, :], ps, AF.Gelu,
                            bias=ap_vec("B1", l * 16 + mc) if flags["b1"] else 0.0,
                            scale=1.0)
                    # W2 in two waves of 2 output chunks (2 live accumulators)
                    sqts = []
                    for wave in range(2):
                        wps = [P["pacc"].tile([128, TT], F32, tag="pa",
                                              name=f"w2ps{m}") for m in range(2)]
                        for kc in range(NC_FF):
                            for m in range(2):
                                nc.tensor.matmul(
                                    wps[m],
                                    w2_t[kc][:, (wave * 2 + m) * 128:(wave * 2 + m + 1) * 128],
                                    hT[kc][:, :],
                                    start=(kc == 0), stop=(kc == NC_FF - 1))
                        for m in range(2):
                            mcD = wave * 2 + m
                            b_ap = ap_vec("B2", l * 4 + mcD) if flags["b2"] else None
                            nc.vector.scalar_tensor_tensor(
                                xf[mcD], wps[m], b_ap if b_ap is not None else 0.0,
                                xf[mcD], ALU.add, ALU.add)
                            sqts.append(mk_sq(xf[mcD]))
                    st = ln_stats(xf, D, sqts=sqts)
                    if defer_ln2:
                        pending_ln2.append((seg, st))
                        return
                    bc = ln_chain(st, D)
                    ln_apply(bc, xf,
                             (lambda c: ap_vec("G2", l * 4 + c)) if flags["g2"] else None,
                             (lambda c: ap_vec("BE2", l * 4 + c)) if flags["be2"] else None,
                             bf_out=xs)

                # ln1 chains split pre/bc: each pre runs under the stats
                # matmuls / ffn block emitted between it and its bc.
                b_stats[0] = wo_stats(0)
                b_stats[1] = wo_stats(1)
                p_ln1[0] = ln_chain_pre(b_stats[0], D)
                b_stats[2] = wo_stats(2)
                ln1_bcapply(0, p_ln1[0])
                p_ln1[1] = ln_chain_pre(b_stats[1], D)
                b_stats[3] = wo_stats(3)
                ln1_bcapply(1, p_ln1[1])
                p_ln1[2] = ln_chain_pre(b_stats[2], D)
                if not (emit_head and l == n_layers - 1):
                    ffn_block(0)
                    ln1_bcapply(2, p_ln1[2])
                    p_ln1[3] = ln_chain_pre(b_stats[3], D)
                    ffn_block(1)
                    ln1_bcapply(3, p_ln1[3])
                    ffn_block(2)
                    ffn_block(3, defer_ln2=True)
                else:
                    # last layer: wavefront the head stages through the FFN
                    # blocks. Every chain is split pre (DVE/ACT) / bc (PE)
                    # with PE-dense work emitted between them, and chains
                    # strictly alternate pre->bc so the bufs=1 mini slots
                    # never stall a pre on an unissued bc. Dummy keep-warm
                    # matmuls (kw) hold the HAM clock at 2.4GHz through the
                    # chain-latency-bound tail.
                    kw_ps = P["ppro"].tile([128, TT], F32, tag="pp", name="kwps")

                    def kw(n=3):
                        for _ in range(n):
                            nc.tensor.matmul(kw_ps, identb, xT[3][0][:, :],
                                             start=True, stop=True)

                    # segs {0,1} head pipelines entirely under ffn(3); segs
                    # {2,3} batch pairwise after it (m/yr bufs=2 lets two
                    # chain pres run back-to-back without waiting the first
                    # chain's broadcast matmuls).
                    s1, s2 = {}, {}
                    p1, p2, b1, b2k = {}, {}, {}, {}
                    ffn_block(0)
                    ln1_bcapply(2, p_ln1[2])
                    p_ln1[3] = ln_chain_pre(b_stats[3], D)
                    ffn_block(1)
                    ln1_bcapply(3, p_ln1[3])
                    ffn_block(2)
                    s1[0] = hp1(0); s1[1] = hp1(1)
                    p1[0] = ln_chain_pre(s1[0], D // 2)
                    p1[1] = ln_chain_pre(s1[1], D // 2)
                    ffn_block(3, defer_ln2=True)
                    (dseg, dst), = pending_ln2; pending_ln2.clear()
                    b1[0] = ln_chain_bc(p1[0]); apply1(0, b1[0])
                    b1[1] = ln_chain_bc(p1[1]); apply1(1, b1[1])
                    ln2p = ln_chain_pre(dst, D)
                    s2[0] = hp2(0); s2[1] = hp2(1)
                    ln2b = ln_chain_bc(ln2p)
                    ln_apply(ln2b, [xF[c][dseg][:, :] for c in range(NC_D)],
                             (lambda c: ap_vec("G2", l * 4 + c)) if flags["g2"] else None,
                             (lambda c: ap_vec("BE2", l * 4 + c)) if flags["be2"] else None,
                             bf_out=[xT[c][dseg][:, :] for c in range(NC_D)])
                    p2[0] = ln_chain_pre(s2[0], D // 4)
                    p2[1] = ln_chain_pre(s2[1], D // 4)
                    s1[2] = hp1(2)
                    b2k[0] = ln_chain_bc(p2[0]); apply2(0, b2k[0])
                    b2k[1] = ln_chain_bc(p2[1]); apply2(1, b2k[1])
                    s1[3] = hp1(3)
                    p1[2] = ln_chain_pre(s1[2], D // 2)
                    p1[3] = ln_chain_pre(s1[3], D // 2)
                    hp3(0); hp3(1)
                    b1[2] = ln_chain_bc(p1[2]); apply1(2, b1[2])
                    b1[3] = ln_chain_bc(p1[3]); apply1(3, b1[3])
                    kw(4)
                    s2[2] = hp2(2); s2[3] = hp2(3)
                    p2[2] = ln_chain_pre(s2[2], D // 4)
                    p2[3] = ln_chain_pre(s2[3], D // 4)
                    kw(4)
                    b2k[2] = ln_chain_bc(p2[2]); apply2(2, b2k[2])
                    b2k[3] = ln_chain_bc(p2[3]); apply2(3, b2k[3])
                    kw(4)
                    hp3(2); hp3(3)

            if not emit_head:
                flush_ln2(n_layers - 1)

            if dbg_x:
                for c in range(NC_D):
                    for s in range(NSEG):
                        nc.sync.dma_start(
                            out=xdbg_d[c * 128:(c + 1) * 128, s * TT:(s + 1) * TT],
                            in_=xT[c][s][:, :])

    nc.compile()
    return nc, extra


# ======================================================================
# Self-contained kernel entry point: takes FULL inputs, shards batch over
# 8 NeuronCores (data-parallel), runs the Bass kernel, gathers output.
# ======================================================================
from concourse.bass_utils import run_bass_kernel_spmd

N_CORES = 8


def make_in_maps(inputs, extra):
    base = dict(extra)
    obs = np.asarray(inputs["observations"], np.float32)
    n_b = obs.shape[0]
    per = n_b // N_CORES
    in_maps = []
    for c in range(N_CORES):
        m = dict(base)
        m["observations"] = np.ascontiguousarray(
            obs[c * per:(c + 1) * per]).astype(BF_NP)
        in_maps.append(m)
    return in_maps, per


def kernel(**inputs):
    inputs = {k: np.asarray(v) for k, v in inputs.items()}
    nc, extra = build(inputs, n_layers=8, emit_head=True, dbg_x=False)
    in_maps, per = make_in_maps(inputs, extra)

    last_err = None
    for attempt in range(4):
        try:
            res = run_bass_kernel_spmd(nc, in_maps, core_ids=list(range(N_CORES)),
                                       trace=False)
            outs = [res.results[c]["OUT"].reshape(per, S, ACT_DIM)
                    for c in range(N_CORES)]
            return np.concatenate(outs, axis=0)
        except Exception as e:  # transient NRT_EXEC_UNIT_UNRECOVERABLE etc.
            last_err = e
            import time as _time
            _time.sleep(3.0 * (attempt + 1))
    raise last_err



# revision 53
# speedup vs baseline: 1.0358x; 1.0007x over previous
"""Transformer policy kernel for TRN2 (Bass/Tile), v4: bf16 matmuls +
fp32 residual + attention-injected projection chunks + V-folded softmax
denominators.
Verified: 2199337 ns HW, rel err 5.35e-3 (v3: 2387458, stub: 2986054).

Per core (data-parallel over batch): BC=8 batches x S=256 -> T=2048 tokens.
D=512 (4 chunks), H=8 heads (HD=64), FF=2048 (16 chunks), L=8 layers.

Design (what actually survived hardware):
- Matmul operands bf16 (host-cast weights): enables Fast Weight Load
  (fp32r gets none -- it was inflating 512-col matmuls 423ns vs 215ns) and
  halves SBUF/DMA. PSUM accumulation fp32.
- Residual stream xF kept in fp32r (storing it bf16 costs ~3e-2 rel err
  over 8 layers); ln_apply writes a bf16 mirror xT via ACT as the matmul
  operand. LN stats read xF directly as fp32r (1 cyc/row at >=256 cols).
- Tableless LN rstd: Quake-seed + 1 Newton step, split DVE/ACT; chains
  further split ln_chain_pre (DVE/ACT) / ln_chain_bc (PE 1-row M/R
  broadcast matmuls) so PE-dense work is emitted between them; m/yr tags
  bufs=2 so two pres can run back-to-back. Stats emit all-sums-then-
  all-sumsq (squares eagerly after each chunk update) so the PE never
  waits on ACT square latency mid-group.
- Softmax denominator folded into V: seg_v holds 8 x (64 feat + ones col),
  the [128,65] AV stationary emits sum-of-exp as psum row 64 (killed 1024
  aux ones-matmuls, ~112us PE, at the cost of FWL on AV: 133->165ns).
  DVE reads of PSUM at partition base 64 return garbage (HW quirk, even
  though 64 is 32-aligned) and gpsimd cannot read PSUM at all -- an ACT
  copy ferries row 64 to SBUF; reciprocal runs 128-lane-wide after the
  gpsimd broadcast.
- Attention pairs ordered c-major so seg_q/k chunk c has no readers after
  pair 2c+1; each pair slot injects one ready PE chunk: attn(0)/(1) soak
  the next-parity qkv chunks, attn(2) soaks wo(0)+wo(1), attn(3) wo(2).
  psmall "sm" slots: scores 2 + merged otpb 1 per pair, so the 4-slot
  round-robin never gates pair p's AV matmuls on pair p+1's exp.
- Whole-matrix single DMAs (rearranged 3D APs) -- dma_start costs ~610ns
  of sync-queue issue each, 36 small DMAs/layer was ~20us; obs DMAs issue
  before the weight loads so input transposes aren't stuck behind them.
- Head (wp1/wp2/wp3) wavefronted through the last layer's FFN blocks with
  pairwise-batched split chains + keep-warm dummy matmuls; scratch in
  dead seg_q/seg_k tiles, not hT (hT is live for later-seg FFNs).

Known rejected/failed directions (measured):
- fp8 DoubleRow W2: 2173915 ns but rel err 2.48e-2 > 2e-2 gate (fp8's ~3%
  per-element noise passes through dot products undiminished).
- DMA cannot read PSUM (dma_start asserts src SBUF/DRAM) -- no DMA ferry.
- Merged [64,2S] division mul + seg_o via 2 DMAs: 2284us vs 2204 (DMA on
  the seg_o critical path delays the injected wo chunks).
- Injecting ln1 bc+apply into attn(3): 2228-2276us (ACT is the attention
  bottleneck; apply's ACT ops delay exps).
- Partition-crossing rearranged SBUF->SBUF DMA trips the sim's conflict
  checker (subtile deps can't cover it) -- use plain 2D slices.
- DVE tensor_tensor needs BOTH SBUF inputs at the same base partition
  (compiler verifier) -- no partition-packing of chain scratch rows.
Observed noise: ~1-in-10 profiled runs land 300-500us high (transient);
one first-exec-after-compile returned corrupt output once (absmax 0.6),
never reproduced across 18+ subsequent execs (flaketest.py).
Remaining levers: attn region is ACT-bound (78% busy: 64 exps/layer at
~690ns each reading PSUM fp32); ~165us PE idle across layers; ~120us
cold-clock penalty (HAM oscillates at the region boundaries); input
stage ~26us idle; tail ~40us. Column-streaming floor ~1690us post-fold.
"""
import math
import contextlib
import numpy as np
import ml_dtypes

import concourse.bass as bass
import concourse.bacc as bacc
import concourse.tile as tile
from concourse import mybir

F32 = mybir.dt.float32
BF16 = mybir.dt.bfloat16
AF = mybir.ActivationFunctionType
ALU = mybir.AluOpType

BF_NP = ml_dtypes.bfloat16

BC = 8
S = 256
T = BC * S
OBS = 96
ACT_DIM = 29
D = 512
H = 8
HD = 64
FF = 2048
NC_D = D // 128
NC_FF = FF // 128
TT = 512
NSEG = T // TT
EPS = 1e-5
L_MAX = 8


def _nz(a):
    return a is not None and bool(np.any(np.asarray(a) != 0))


def _ng(a):
    return a is not None and bool(np.any(np.asarray(a) != 1))


def build(inputs, n_layers=8, emit_head=True, dbg_x=False):
    """inputs: dict of full np arrays (reference naming). Returns (nc, extra_in_map)."""
    nc = bacc.Bacc("TRN2", target_bir_lowering=False, debug=False)

    flags = dict(
        bin_=_nz(inputs["b_in"]), gin=_ng(inputs["g_in"]), bein=_nz(inputs["be_in"]),
        bq=_nz(inputs["bq"]), bk=_nz(inputs["bk"]), bv=_nz(inputs["bv"]), bo=_nz(inputs["bo"]),
        g1=_ng(inputs["g1"]), be1=_nz(inputs["be1"]), b1=_nz(inputs["b1"]), b2=_nz(inputs["b2"]),
        g2=_ng(inputs["g2"]), be2=_nz(inputs["be2"]),
        bp1=_nz(inputs["bp1"]), gp1=_ng(inputs["gp1"]), bep1=_nz(inputs["bep1"]),
        bp2=_nz(inputs["bp2"]), gp2=_ng(inputs["gp2"]), bep2=_nz(inputs["bep2"]),
        bp3=_nz(inputs["bp3"]), asc=_ng(inputs["action_scale"]), abi=_nz(inputs["action_bias"]),
    )

    def din(name, shape, dt=BF16):
        return nc.dram_tensor(name, shape, dt, kind="ExternalInput").ap()

    F32R = mybir.dt.float32r
    obs_d = din("observations", (BC, S, OBS))
    win_d = din("W_in", (OBS, D))
    wq_d = din("Wq", (L_MAX, D, D)); wk_d = din("Wk", (L_MAX, D, D))
    wv_d = din("Wv", (L_MAX, D, D)); wo_d = din("Wo", (L_MAX, D, D))
    w1_d = din("W1", (L_MAX, D, FF)); w2_d = din("W2", (L_MAX, FF, D))
    wp1_d = din("Wp1", (D, D // 2)); wp2_d = din("Wp2", (D // 2, D // 4))
    wp3_d = din("Wp3", (D // 4, ACT_DIM))
    identb_d = din("IDENTB", (128, 128))
    identf_d = din("IDENTF", (128, 128), F32)
    ones_d = din("ONES", (128, 8))
    onesf_d = din("ONESF", (128, 128), F32R)
    pet_d = din("PET", (D, S))
    out_d = nc.dram_tensor("OUT", (T, ACT_DIM), F32, kind="ExternalOutput").ap()
    if dbg_x:
        xdbg_d = nc.dram_tensor("XDBG", (D, T), BF16, kind="ExternalOutput").ap()

    extra = {
        "IDENTB": np.eye(128, dtype=BF_NP),
        "IDENTF": np.eye(128, dtype=np.float32),
        "ONES": np.ones((128, 8), BF_NP),
        "ONESF": np.ones((128, 128), np.float32),
    }
    pos = np.arange(S, dtype=np.float32)[:, None]
    div = np.exp(np.arange(0, D, 2, dtype=np.float32) * (-math.log(10000.0) / D))
    pe = np.zeros((S, D), dtype=np.float32)
    pe[:, 0::2] = np.sin(pos * div)
    pe[:, 1::2] = np.cos(pos * div)
    extra["PET"] = np.ascontiguousarray(pe.T).astype(BF_NP)
    for k in ["W_in", "Wq", "Wk", "Wv", "Wo", "W1", "W2", "Wp1", "Wp2", "Wp3"]:
        extra[k] = np.ascontiguousarray(np.asarray(inputs[k], np.float32)).astype(BF_NP)

    # per-feature bias/gain vectors (feature-major [128, n]) -- only emitted
    # when the corresponding values are nontrivial (not for this problem).
    def vec_tensor(name, arr):
        a = np.asarray(arr, np.float32).reshape(-1)
        n = a.size // 128
        extra[name] = np.ascontiguousarray(a.reshape(n, 128).T)
        return din(name, (128, n), F32)

    dv = {}
    for key, nm in [("bq", "BQ"), ("bk", "BK"), ("bo", "BO"), ("b1", "B1"), ("b2", "B2"),
                    ("b_in", "BIN"), ("g_in", "GIN"), ("be_in", "BEIN"),
                    ("g1", "G1"), ("be1", "BE1"), ("g2", "G2"), ("be2", "BE2"),
                    ("bp1", "BP1"), ("gp1", "GP1"), ("bep1", "BEP1"),
                    ("bp2", "BP2"), ("gp2", "GP2"), ("bep2", "BEP2")]:
        fkey = {"b_in": "bin_", "g_in": "gin", "be_in": "bein"}.get(key, key)
        if flags[fkey]:
            dv[nm] = vec_tensor(nm + "v", inputs[key])
    if flags["bv"]:
        extra["BVr"] = np.asarray(inputs["bv"], np.float32).astype(BF_NP).reshape(L_MAX, D)
        dv["BV"] = din("BVr", (L_MAX, D))

    def vec29(name, arr):
        a = np.zeros((128, 1), np.float32)
        a[:ACT_DIM, 0] = np.asarray(arr, np.float32).reshape(-1)
        extra[name] = a
        return din(name, (128, 1), F32)
    if flags["bp3"]:
        dv["BP3"] = vec29("BP3v", inputs["bp3"])
    if flags["asc"]:
        dv["ASC"] = vec29("ASCv", inputs["action_scale"])
    if flags["abi"]:
        dv["ABI"] = vec29("ABIv", inputs["action_bias"])

    scale = 1.0 / math.sqrt(HD)

    with tile.TileContext(nc) as tc:
        with contextlib.ExitStack() as ctx:
            P = {}
            P["persist"] = ctx.enter_context(tc.tile_pool(name="persist", bufs=1))
            P["wpool"] = ctx.enter_context(tc.tile_pool(name="wpool", bufs=1))
            P["xpool"] = ctx.enter_context(tc.tile_pool(name="xpool", bufs=1))
            P["segt"] = ctx.enter_context(tc.tile_pool(name="segt", bufs=1))
            P["hpool"] = ctx.enter_context(tc.tile_pool(name="hpool", bufs=1))
            P["exps"] = ctx.enter_context(tc.tile_pool(name="exps", bufs=2))
            P["sq"] = ctx.enter_context(tc.tile_pool(name="sq", bufs=2))
            P["scratch"] = ctx.enter_context(tc.tile_pool(name="scratch", bufs=2))
            P["mini"] = ctx.enter_context(tc.tile_pool(name="mini", bufs=1))
            P["rbs"] = ctx.enter_context(tc.tile_pool(name="rbs", bufs=2))
            P["rec"] = ctx.enter_context(tc.tile_pool(name="rec", bufs=1))
            P["ppro"] = ctx.enter_context(tc.tile_pool(name="ppro", bufs=2, space="PSUM"))
            P["pacc"] = ctx.enter_context(tc.tile_pool(name="pacc", bufs=2, space="PSUM"))
            P["psmall"] = ctx.enter_context(tc.tile_pool(name="psmall", bufs=4, space="PSUM"))

            # ---------------- constants ----------------
            identb = P["persist"].tile([128, 128], BF16, tag="identb")
            nc.sync.dma_start(out=identb, in_=identb_d[:, :])
            identf = P["persist"].tile([128, 128], F32, tag="identf")
            nc.sync.dma_start(out=identf, in_=identf_d[:, :])
            ones = P["persist"].tile([128, 8], BF16, tag="ones")
            nc.sync.dma_start(out=ones, in_=ones_d[:, :])
            ones_col = ones[:, 0:1]
            F32R = mybir.dt.float32r
            onesf = P["persist"].tile([128, 128], F32R, tag="onesf")
            nc.sync.dma_start(out=onesf, in_=onesf_d[:, :])
            onesf_col = onesf[:, 0:1]
            onesf_row = onesf[0:1, :]

            peT = P["persist"].tile([128, NC_D * S], BF16, tag="peT")
            nc.sync.dma_start(out=peT.rearrange("p (c s) -> p c s", s=S),
                              in_=pet_d.rearrange("(c p) s -> p c s", p=128))

            vt = {}
            for nm, d in dv.items():
                if nm == "BV":
                    t = P["persist"].tile([1, L_MAX * D], BF16, tag="c_BV")
                    for l in range(L_MAX):
                        nc.sync.dma_start(out=t[:, l * D:(l + 1) * D], in_=d[l:l + 1, :])
                else:
                    t = P["persist"].tile([128, d.shape[1]], F32, tag=f"c_{nm}")
                    nc.sync.dma_start(out=t, in_=d[:, :])
                vt[nm] = t

            def ap_vec(nm, idx):
                t = vt.get(nm)
                return t[:, idx:idx + 1] if t is not None else None

            # ---------------- big tiles ----------------
            # xF: fp32 residual stream (rounding the residual to bf16 costs
            # ~3e-2 rel err over 8 layers); xT: bf16 mirror fed to matmuls.
            xF = [[P["xpool"].tile([128, TT], F32R, tag=f"xF{c}_{s}", name=f"xF{c}_{s}")
                   for s in range(NSEG)] for c in range(NC_D)]
            xT = [[P["xpool"].tile([128, TT], BF16, tag=f"xT{c}_{s}", name=f"xT{c}_{s}")
                   for s in range(NSEG)] for c in range(NC_D)]
            # double-buffered q/k/v (indexed seg%2), per-seg o
            seg_q = [[P["segt"].tile([128, TT], BF16, tag=f"sq{c}_{b}", name=f"sq{c}_{b}")
                      for c in range(NC_D)] for b in range(2)]
            seg_k = [[P["segt"].tile([128, TT], BF16, tag=f"sk{c}_{b}", name=f"sk{c}_{b}")
                      for c in range(NC_D)] for b in range(2)]
            # seg_v carries 8 heads x (64 feat + 1 ones col): the ones col
            # folds the softmax denominator into the AV matmul (row 64 of
            # the [65, S] AV output = sum of exp), killing the aux ones
            # matmuls (4 per pair, ~112us of PE across the kernel).
            seg_v = [[P["segt"].tile([128, H * (HD + 1)], BF16, tag=f"sv{c}_{b}",
                                     name=f"sv{c}_{b}")
                      for c in range(NC_D)] for b in range(2)]
            seg_o = [[P["segt"].tile([128, TT], BF16, tag=f"so{c}_{s}", name=f"so{c}_{s}")
                      for c in range(NC_D)] for s in range(NSEG)]
            hT = [P["hpool"].tile([128, TT], BF16, tag=f"hT{m}", name=f"hT{m}")
                  for m in range(NC_FF)]

            # weights: qkvo+w1 double-buffered, w2 single (prefetched per layer)
            # each matrix is ONE [128, NC*D] tile (chunk-major cols) so a
            # whole matrix loads in a single DMA: the sync queue costs
            # ~610ns per dma_start, so 36 small DMAs/layer was ~20us of
            # issue serialization.
            wq_b = [P["wpool"].tile([128, NC_D * D], BF16, tag=f"wq_{b}", name=f"wq_{b}")
                    for b in range(2)]
            wk_b = [P["wpool"].tile([128, NC_D * D], BF16, tag=f"wk_{b}", name=f"wk_{b}")
                    for b in range(2)]
            wv_b = [P["wpool"].tile([128, NC_D * D], BF16, tag=f"wv_{b}", name=f"wv_{b}")
                    for b in range(2)]
            wo_b = [P["wpool"].tile([128, NC_D * D], BF16, tag=f"wo_{b}", name=f"wo_{b}")
                    for b in range(2)]
            wq_t = [[wq_b[b][:, k * D:(k + 1) * D] for k in range(NC_D)] for b in range(2)]
            wk_t = [[wk_b[b][:, k * D:(k + 1) * D] for k in range(NC_D)] for b in range(2)]
            wv_t = [[wv_b[b][:, k * D:(k + 1) * D] for k in range(NC_D)] for b in range(2)]
            wo_t = [[wo_b[b][:, k * D:(k + 1) * D] for k in range(NC_D)] for b in range(2)]
            w1_bt = P["wpool"].tile([128, NC_D * FF], BF16, tag="w1", name="w1t")
            w1_t = [w1_bt[:, k * FF:(k + 1) * FF] for k in range(NC_D)]
            w2_bt = P["wpool"].tile([128, NC_FF * D], BF16, tag="w2", name="w2t")
            w2_t = [w2_bt[:, k * D:(k + 1) * D] for k in range(NC_FF)]
            # head/input weights: win aliases w2_t[0] (w2 of layer 0 is
            # loaded only after the input-stage projections are emitted)
            win_t = w2_t[0][:, :]
            wp1_t = [P["wpool"].tile([128, D // 2], BF16, tag=f"wp1{k}", name=f"wp1{k}")
                     for k in range(NC_D)]
            wp2_t = [P["wpool"].tile([128, D // 4], BF16, tag=f"wp2{k}", name=f"wp2{k}")
                     for k in range(2)]
            wp3_t = P["wpool"].tile([128, 32], BF16, tag="wp3")

            def load_layer_weights(lx, b):
                for dst, srcd in [(wq_b[b], wq_d), (wk_b[b], wk_d),
                                   (wv_b[b], wv_d), (wo_b[b], wo_d)]:
                    nc.sync.dma_start(
                        out=dst.rearrange("p (k j) -> p k j", j=D),
                        in_=srcd[lx].rearrange("(k p) j -> p k j", p=128))

            def load_w12(lx):
                nc.sync.dma_start(out=w1_bt.rearrange("p (k j) -> p k j", j=FF),
                                  in_=w1_d[lx].rearrange("(k p) j -> p k j", p=128))
                nc.sync.dma_start(out=w2_bt.rearrange("p (k j) -> p k j", j=D),
                                  in_=w2_d[lx].rearrange("(k p) j -> p k j", p=128))

            # ---------------- layernorm (split into stats / chain / apply) ----
            def mk_sq(chunk_ap):
                sqt = P["sq"].tile([128, TT], F32R, tag="sqt")
                nc.scalar.square(sqt, chunk_ap)
                return sqt

            def ln_stats(chunk_aps, nfeat, sqts=None):
                """stats over fp32r chunks: squares on ACT, sums via ones
                matmuls (fp32r, 1 cyc/row at >=256 cols). All sums matmuls
                go first so the PE isn't gated on ACT square latency."""
                nch = len(chunk_aps)
                sums = P["psmall"].tile([1, TT], F32, tag="sm", name="sums")
                sumsq = P["psmall"].tile([1, TT], F32, tag="sm", name="sumsq")
                if sqts is None:
                    sqts = [mk_sq(chunk_aps[c]) for c in range(nch)]
                for c in range(nch):
                    nc.tensor.matmul(sums, onesf_col, chunk_aps[c],
                                     start=(c == 0), stop=(c == nch - 1))
                for c in range(nch):
                    nc.tensor.matmul(sumsq, onesf_col, sqts[c],
                                     start=(c == 0), stop=(c == nch - 1))
                return sums, sumsq

            def ln_chain_pre(stats, nfeat, n_newton=1):
                """mean + rstd for a token-column LN, tableless: rstd via
                Quake-seed + Newton, split across ACT (Copy/Square, in every
                table) and DVE (muls + int seed). Returns (m, yr) row
                vectors; ln_chain_bc broadcasts them on the PE."""
                sums, sumsq = stats
                I32 = mybir.dt.int32
                m = P["mini"].tile([1, TT], F32R, tag="m", bufs=2)
                nc.scalar.mul(m, sums, 1.0 / nfeat)
                e2 = P["mini"].tile([1, TT], F32, tag="e2")
                nc.scalar.activation(e2, sumsq, AF.Copy, bias=float(EPS),
                                     scale=1.0 / nfeat)
                msq = P["mini"].tile([1, TT], F32, tag="msq")
                nc.scalar.square(msq, m)
                nc.vector.tensor_sub(e2, e2, msq)  # e2 = var + eps
                y = P["mini"].tile([1, TT], F32, tag="y")
                nc.vector.tensor_scalar(out=y.bitcast(I32), in0=e2.bitcast(I32),
                                        scalar1=1, scalar2=None,
                                        op0=ALU.logical_shift_right)
                nc.vector.tensor_scalar(out=y.bitcast(I32), in0=y.bitcast(I32),
                                        scalar1=0x5F3759DF, scalar2=-1,
                                        op0=ALU.subtract, op1=ALU.mult)
                # last Newton step writes a fresh fp32r tile: fp32r matmul
                # operands must come from a producer that rounds to fp32r,
                # which the int-ALU seed ops above don't.
                yr = P["mini"].tile([1, TT], F32R, tag="yr", bufs=2)
                for it in range(n_newton):
                    a = P["mini"].tile([1, TT], F32, tag="nta")
                    nc.vector.tensor_mul(a, y, y)
                    nc.vector.tensor_mul(a, a, e2)
                    nc.scalar.activation(a, a, AF.Copy, bias=1.5, scale=-0.5)
                    nc.vector.tensor_mul(yr if it == n_newton - 1 else y, y, a)
                return m, yr

            def ln_chain_bc(pre):
                # broadcast M/R across partitions on the PE (gpsimd broadcast
                # latency straggles to 4us+ and stalled the FFN start). The
                # ones stationary row is picked at the moving operand's base
                # partition (matmul requires matching bases).
                m, yr = pre
                bm, br = m.base_partition(), yr.base_partition()
                M = P["pacc"].tile([128, TT], F32, tag="pa", name="Mb")
                nc.tensor.matmul(M, onesf[bm:bm + 1, :], m, start=True, stop=True)
                R = P["pacc"].tile([128, TT], F32, tag="pa", name="Rb")
                nc.tensor.matmul(R, onesf[br:br + 1, :], yr, start=True, stop=True)
                return M, R

            def ln_chain(stats, nfeat, n_newton=1):
                return ln_chain_bc(ln_chain_pre(stats, nfeat, n_newton))

            def ln_apply(bc, chunk_aps, g_fn=None, b_fn=None, gelu=False,
                         bf_out=None):
                """normalize fp32r chunks in place; optionally gelu; optionally
                write a bf16 mirror (the matmul operand) via ACT."""
                M, R = bc
                for c in range(len(chunk_aps)):
                    xc = chunk_aps[c]
                    g_ap = g_fn(c) if g_fn is not None else None
                    b_ap = b_fn(c) if b_fn is not None else None
                    nc.vector.tensor_sub(xc, xc, M)
                    if g_ap is not None:
                        nc.vector.scalar_tensor_tensor(xc, xc, g_ap, R,
                                                       ALU.mult, ALU.mult)
                    else:
                        nc.vector.tensor_mul(xc, xc, R)
                    if gelu:
                        nc.scalar.activation(xc, xc, AF.Gelu,
                                             bias=b_ap if b_ap is not None else 0.0,
                                             scale=1.0)
                    elif b_ap is not None:
                        nc.scalar.activation(xc, xc, AF.Identity, bias=b_ap,
                                             scale=1.0)
                    if bf_out is not None:
                        nc.scalar.copy(bf_out[c], xc)

            # ---------------- projections ----------------
            def proj_fm(w_tiles, in_aps, out_aps, bias_fn=None, kpart=128,
                        epi="act"):
                """feature-major projection: out[mc] = W.T @ in (+bias).
                Epilogue copy on ACT by default (DVE is the busier engine)."""
                n_out = len(out_aps)
                n_in = len(in_aps)
                for mc in range(n_out):
                    ps = P["ppro"].tile([128, TT], F32, tag="pp")
                    for kc in range(n_in):
                        nc.tensor.matmul(
                            ps, w_tiles[kc][0:kpart, mc * 128:(mc + 1) * 128],
                            in_aps[kc][0:kpart, :],
                            start=(kc == 0), stop=(kc == n_in - 1))
                    b_ap = bias_fn(mc) if bias_fn is not None else None
                    if b_ap is None and epi == "dve":
                        nc.vector.tensor_copy(out_aps[mc], ps)
                    elif b_ap is None:
                        nc.scalar.copy(out_aps[mc], ps)
                    else:
                        nc.scalar.activation(out_aps[mc], ps, AF.Identity,
                                             bias=b_ap, scale=1.0)

            # ---------------- attention ----------------
            def qkv_chunks(seg, l, wb):
                """return the q/k/v projection of one seg as 12 single-psum
                chunk callables: [q0,k0,q1,k1,q2,k2] (safe to inject into
                the PREVIOUS seg-parity attention at pair slots 2.. with
                c-major pair order) and [q3,k3,v0..v3] (safe once that
                attention is fully emitted)."""
                db = seg % 2
                xs = [xT[c][seg][:, :] for c in range(NC_D)]

                def q_chunk(mc):
                    def f():
                        ps = P["ppro"].tile([128, TT], F32, tag="pp")
                        for kc in range(NC_D):
                            nc.tensor.matmul(
                                ps, wq_t[wb][kc][:, mc * 128:(mc + 1) * 128],
                                xs[kc], start=(kc == 0), stop=(kc == NC_D - 1))
                        b_ap = ap_vec("BQ", l * 4 + mc) if flags["bq"] else None
                        if b_ap is None:
                            nc.scalar.copy(seg_q[db][mc][:, :], ps)
                        else:
                            nc.scalar.activation(seg_q[db][mc][:, :], ps,
                                                 AF.Identity, bias=b_ap, scale=1.0)
                    return f

                def k_chunk(mc):
                    def f():
                        ps = P["ppro"].tile([128, TT], F32, tag="pp")
                        for kc in range(NC_D):
                            nc.tensor.matmul(
                                ps, wk_t[wb][kc][:, mc * 128:(mc + 1) * 128],
                                xs[kc], start=(kc == 0), stop=(kc == NC_D - 1))
                        b_ap = ap_vec("BK", l * 4 + mc) if flags["bk"] else None
                        if b_ap is None:
                            nc.vector.tensor_copy(seg_k[db][mc][:, :], ps)
                        else:
                            nc.scalar.activation(seg_k[db][mc][:, :], ps,
                                                 AF.Identity, bias=b_ap, scale=1.0)
                    return f

                def v_chunk(ts):
                    def f():
                        vp = P["ppro"].tile([128, D], F32, tag="pp")
                        for kc in range(NC_D):
                            nc.tensor.matmul(
                                vp, xT[kc][seg][:, ts * 128:(ts + 1) * 128],
                                wv_t[wb][kc],
                                start=(kc == 0), stop=(kc == NC_D - 1) and not flags["bv"])
                        if flags["bv"]:
                            nc.tensor.matmul(vp, ones[0:1, 0:128],
                                             vt["BV"][:, l * D:(l + 1) * D],
                                             start=False, stop=True)
                        nc.vector.tensor_copy(
                            seg_v[db][ts].rearrange("p (h g) -> p h g", g=HD + 1)[:, :, 0:HD],
                            vp.rearrange("p (h f) -> p h f", f=HD))
                    return f

                early = [q_chunk(0), k_chunk(0), q_chunk(1), k_chunk(1),
                         q_chunk(2), k_chunk(2)]
                late = [q_chunk(3), k_chunk(3)] + [v_chunk(ts) for ts in range(4)]
                return early, late

            def emit_qkv(seg, l, wb):
                early, late = qkv_chunks(seg, l, wb)
                for f in early + late:
                    f()

            def emit_attn(seg, inject=None):
                """softmax attention for one segment (2 batches x 4 head pairs).
                One-pair lookahead: pair p+1's score matmuls are emitted before
                pair p's AV matmuls so the PE never waits on Exp. Pair order is
                c-major so seg_q/k chunk c has no readers after pair 2c+1 --
                lets the next-parity qkv chunks inject early. `inject` is a
                list of callables emitting ready PE work, consumed one per
                pair slot to fill the exp-latency bubbles."""
                db = seg % 2
                pairs = [(b2, c) for c in range(NC_D) for b2 in range(2)]
                inject = list(inject) if inject else []
                st = {}

                def do_inject(n=1):
                    for _ in range(n):
                        if inject:
                            inject.pop(0)()

                def scores(p):
                    b2, c = pairs[p]
                    bcol = b2 * S
                    scps, ess = [], []
                    for hh in range(2):
                        roff = hh * HD
                        scp = P["psmall"].tile([128, 2 * S], F32, tag="sm",
                                               name=f"scp{hh}")
                        for kc in range(2):
                            nc.tensor.matmul(
                                scp[:, kc * S:(kc + 1) * S],
                                seg_k[db][c][roff:roff + HD,
                                             bcol + kc * 128: bcol + (kc + 1) * 128],
                                seg_q[db][c][roff:roff + HD, bcol:bcol + S],
                                start=True, stop=True)
                        scps.append(scp)
                    for hh in range(2):
                        esh = P["exps"].tile([128, 2 * S], BF16, tag=f"es{hh}",
                                             name=f"es{hh}")
                        nc.scalar.activation(esh, scps[hh], AF.Exp, bias=0.0,
                                             scale=scale)
                        ess.append(esh)
                    st[p] = ess

                def avpart(p):
                    # v1-proven AV layout: both heads' AV outputs at base
                    # partition 0 (PSUM col-group 3 -- output partitions
                    # 96:127 -- is a buggy PE quadrant, so no col-tiling);
                    # the odd head reaches seg_o partitions 64:128 via an
                    # SBUF->SBUF partition-shift DMA. aux+otp are 2 "sm"
                    # slots (not 3) so the 4-slot round-robin never makes
                    # pair p's AV matmuls wait on pair p+1's exp.
                    b2, c = pairs[p]
                    bcol = b2 * S
                    ess = st.pop(p)
                    otpb = P["psmall"].tile([HD + 1, 2 * S], F32, tag="sm", name="otpb")
                    otp = [otpb[0:HD, hh * S:(hh + 1) * S] for hh in range(2)]
                    for hh in range(2):
                        h = 2 * c + hh
                        for kc in range(2):
                            nc.tensor.matmul(
                                otpb[0:HD + 1, hh * S:(hh + 1) * S],
                                seg_v[db][b2 * 2 + kc][:, h * (HD + 1):(h + 1) * (HD + 1)],
                                ess[hh][:, kc * S:(kc + 1) * S],
                                start=(kc == 0), stop=(kc == 1))
                    # denom row 64 of the AV psum: DVE reads at partition
                    # base 64 of PSUM return garbage (HW quirk) and gpsimd
                    # can't read PSUM at all, so a sync DMA ferries the row
                    # to SBUF (ACT is the attention-phase bottleneck); recip
                    # runs 128-lane-wide after the broadcast. One wide mul
                    # divides both heads; one 3D-AP DMA scatters the halves
                    # into seg_o partitions 0:64 / 64:128.
                    rec = P["rec"].tile([1, 2 * S], F32, tag="rec")
                    nc.scalar.copy(rec, otpb[HD:HD + 1, :])
                    rbs = P["rbs"].tile([128, 2 * S], F32, tag="rbs")
                    nc.gpsimd.partition_broadcast(rbs, rec)
                    nc.vector.reciprocal_approx_fast(out=rbs, in_=rbs)
                    nc.vector.tensor_mul(
                        seg_o[seg][c][0:HD, bcol:bcol + S],
                        otp[0], rbs[0:HD, 0:S])
                    otmp = P["rbs"].tile([HD, S], BF16, tag="otmp")
                    nc.vector.tensor_mul(otmp, otp[1], rbs[0:HD, S:2 * S])
                    nc.sync.dma_start(out=seg_o[seg][c][HD:128, bcol:bcol + S],
                                      in_=otmp)

                scores(0)
                for p in range(1, 8):
                    scores(p)
                    avpart(p - 1)
                    # pair p's q/k chunk frees at pair 2c+1; injected chunk
                    # c's epilogue waits scores(2c+1), so keep c <= (p-2)//2
                    # to avoid parking the ACT/DVE queue on a long wait.
                    do_inject(1)
                avpart(7)
                do_inject(len(inject))

            # ---------------- input stage ----------------
            # ---- PE warm-up: ~4.5us of back-to-back matmuls releases the HAM
            # clock throttle (cold PE runs at 1.2 instead of 2.4 GHz) before
            # the thin-PE input stage and layer 0 begin.
            warm_ps = P["ppro"].tile([128, 128], F32, tag="pp", name="warmps")
            for _ in range(40):
                nc.tensor.matmul(warm_ps, identb, identb, start=True, stop=True)
            warm_out = P["scratch"].tile([128, 128], BF16, tag="warmo")
            nc.vector.tensor_copy(warm_out, warm_ps)

            # obs DMAs batched (1/seg) and issued before the weight loads so
            # the transposes aren't stuck behind ~16us of sync-queue issue.
            obs_flat = obs_d.rearrange("b s f -> (b s) f")
            ot_segs = []
            for seg in range(NSEG):
                # stage in dead seg_v tiles (free until layer-0 qkv)
                ot = seg_v[seg % 2][seg // 2][:, 0:4 * OBS]
                nc.sync.dma_start(
                    out=ot.rearrange("p (ts f) -> p ts f", f=OBS),
                    in_=obs_flat[seg * TT:(seg + 1) * TT]
                        .rearrange("(ts p) f -> p ts f", p=128))
                ot_segs.append(ot)
            nc.sync.dma_start(out=win_t[0:OBS, :], in_=win_d[:, :])
            in_stats = {}
            for seg in range(NSEG):
                obsT = seg_k[seg % 2][seg // 2]  # [96, 512] region staging
                for ts in range(4):
                    tp = P["psmall"].tile([OBS, 128], BF16, tag="sm", name="tpin")
                    nc.tensor.transpose(
                        tp, ot_segs[seg][:, ts * OBS:(ts + 1) * OBS], identb)
                    nc.vector.tensor_copy(obsT[0:OBS, ts * 128:(ts + 1) * 128], tp)
                xf = [xF[c][seg][:, :] for c in range(NC_D)]
                proj_fm([win_t], [obsT[:, :]], xf,
                        (lambda mc: ap_vec("BIN", mc)) if flags["bin_"] else None,
                        kpart=OBS)
                in_stats[seg] = ln_stats(xf, D)
            load_layer_weights(0, 0)
            load_w12(0)  # w2_t[0] aliases win; DMA waits the proj reads above
            # ones columns of seg_v (col h*65+64): written once, after the
            # obs staging reads; v-projection epilogues never touch them.
            for b in range(2):
                for ts in range(4):
                    nc.vector.tensor_copy(
                        seg_v[b][ts].rearrange("p (h g) -> p h g", g=HD + 1)[:, :, HD:HD + 1],
                        ones[:, 0:H].unsqueeze(2))
            in_bc = {}
            for seg in range(NSEG):
                in_bc[seg] = ln_chain(in_stats[seg], D)
            for seg in range(NSEG):
                xf = [xF[c][seg][:, :] for c in range(NC_D)]
                ln_apply(in_bc[seg], xf,
                         (lambda c: ap_vec("GIN", c)) if flags["gin"] else None,
                         (lambda c: ap_vec("BEIN", c)) if flags["bein"] else None,
                         gelu=True)
                for c in range(NC_D):
                    xc = xf[c]
                    nc.vector.tensor_add(
                        xc.rearrange("p (b s) -> p b s", s=S),
                        xc.rearrange("p (b s) -> p b s", s=S),
                        peT[:, c * S:(c + 1) * S].unsqueeze(1)
                           .broadcast_to([128, TT // S, S]))
                    nc.scalar.copy(xT[c][seg][:, :], xc)

            # ---------------- layers ----------------
            # ln2 chain of the last couple of segs is deferred into the next
            # layer's attention phase so it hides under qkv/attention PE work
            # instead of stalling the layer tail.
            pending_ln2 = []

            def flush_ln2(l_prev):
                for seg, st in pending_ln2:
                    xf = [xF[c][seg][:, :] for c in range(NC_D)]
                    bc = ln_chain(st, D)
                    ln_apply(bc, xf,
                             (lambda c: ap_vec("G2", l_prev * 4 + c)) if flags["g2"] else None,
                             (lambda c: ap_vec("BE2", l_prev * 4 + c)) if flags["be2"] else None,
                             bf_out=[xT[c][seg][:, :] for c in range(NC_D)])
                pending_ln2.clear()

            # ---------------- head helpers (per-seg, interleaved into the
            # last layer's FFN blocks so the LN chains hide under PE work
            # and the PE never goes cold at the kernel tail) --------------
            def hp1(seg):
                xs = [xT[c][seg][:, :] for c in range(NC_D)]
                y1f = [xF[mc][seg][:, :] for mc in range(2)]
                proj_fm(wp1_t, xs, y1f,
                        (lambda mc: ap_vec("BP1", mc)) if flags["bp1"] else None)
                return ln_stats(y1f, D // 2)

            def apply1(seg, bc):
                y1f = [xF[mc][seg][:, :] for mc in range(2)]
                y1b = [seg_q[seg % 2][mc][:, :] for mc in range(2)]
                ln_apply(bc, y1f,
                         (lambda c: ap_vec("GP1", c)) if flags["gp1"] else None,
                         (lambda c: ap_vec("BEP1", c)) if flags["bep1"] else None,
                         gelu=True, bf_out=y1b)

            def hp2(seg):
                y1b = [seg_q[seg % 2][mc][:, :] for mc in range(2)]
                y2f = [xF[2][seg][:, :]]
                proj_fm(wp2_t, y1b, y2f,
                        (lambda mc: ap_vec("BP2", 0)) if flags["bp2"] else None)
                return ln_stats(y2f, D // 4)

            def apply2(seg, bc):
                y2f = [xF[2][seg][:, :]]
                y2b = [seg_k[seg % 2][0][:, :]]
                ln_apply(bc, y2f,
                         (lambda c: ap_vec("GP2", 0)) if flags["gp2"] else None,
                         (lambda c: ap_vec("BEP2", 0)) if flags["bep2"] else None,
                         gelu=True, bf_out=y2b)

            def hp3(seg):
                y2b = seg_k[seg % 2][0][:, :]
                actp = P["psmall"].tile([ACT_DIM, TT], F32, tag="sm", name="actp")
                nc.tensor.matmul(actp, wp3_t[:, 0:ACT_DIM], y2b,
                                 start=True, stop=True)
                actT = P["mini"].tile([ACT_DIM, TT], F32, tag="actT")
                nc.scalar.activation(actT[0:ACT_DIM, :], actp, AF.Tanh,
                                     bias=vt["BP3"][0:ACT_DIM, 0:1] if flags["bp3"] else 0.0,
                                     scale=1.0)
                if flags["asc"] or flags["abi"]:
                    nc.scalar.activation(
                        actT[0:ACT_DIM, :], actT[0:ACT_DIM, :], AF.Identity,
                        bias=vt["ABI"][0:ACT_DIM, 0:1] if flags["abi"] else 0.0,
                        scale=vt["ASC"][0:ACT_DIM, 0:1] if flags["asc"] else 1.0)
                for ts in range(4):
                    tp = P["ppro"].tile([128, ACT_DIM], F32, tag="pp", name="tpo")
                    nc.tensor.transpose(tp, actT[0:ACT_DIM, ts * 128:(ts + 1) * 128],
                                        identf[0:ACT_DIM, 0:ACT_DIM])
                    ob = P["scratch"].tile([128, ACT_DIM], F32, tag="ob")
                    nc.vector.tensor_copy(ob, tp)
                    nc.sync.dma_start(
                        out=out_d[seg * TT + ts * 128: seg * TT + (ts + 1) * 128, :],
                        in_=ob)

            def load_head_weights():
                for k in range(NC_D):
                    nc.sync.dma_start(out=wp1_t[k], in_=wp1_d[k * 128:(k + 1) * 128, :])
                for k in range(2):
                    nc.sync.dma_start(out=wp2_t[k], in_=wp2_d[k * 128:(k + 1) * 128, :])
                nc.sync.dma_start(out=wp3_t[:, 0:ACT_DIM], in_=wp3_d[:, :])

            for l in range(n_layers):
                wb = l % 2
                if l + 1 < n_layers:
                    load_layer_weights(l + 1, 1 - wb)
                if l > 0:
                    load_w12(l)
                if emit_head and l == n_layers - 1:
                    load_head_weights()

                # phase B helpers (defined first; phase A injects wo chunks)
                def wo_chunks(seg):
                    xf = [xF[c][seg][:, :] for c in range(NC_D)]

                    def chunk(mc):
                        def f():
                            ps = P["ppro"].tile([128, TT], F32, tag="pp")
                            for kc in range(NC_D):
                                nc.tensor.matmul(
                                    ps, wo_t[wb][kc][:, mc * 128:(mc + 1) * 128],
                                    seg_o[seg][kc][:, :],
                                    start=(kc == 0), stop=(kc == NC_D - 1))
                            b_ap = ap_vec("BO", l * 4 + mc) if flags["bo"] else None
                            nc.vector.scalar_tensor_tensor(
                                xf[mc], ps, b_ap if b_ap is not None else 0.0,
                                xf[mc], ALU.add, ALU.add)
                        return f
                    return [chunk(mc) for mc in range(NC_D)]

                def wo_stats(seg):
                    xf = [xF[c][seg][:, :] for c in range(NC_D)]
                    sqts = [mk_sq(xf[mc]) for mc in range(NC_D)]
                    return ln_stats(xf, D, sqts=sqts)

                # phase A: qkv + attention; attention pair slots soak the
                # next-parity qkv chunks (attn 0/1) and the wo chunks of
                # completed segs (attn 2/3), keeping the PE fed through the
                # exp/broadcast serial chains. The deferred ln2 chain runs
                # under qkv PE work and must precede attn(s0) so its psmall
                # stats banks free up before attention recycles them.
                emit_qkv(0, l, wb)
                emit_qkv(1, l, wb)
                flush_ln2(l - 1)
                e2 = qkv_chunks(2, l, wb)
                emit_attn(0, inject=e2[0] + e2[1])
                e3 = qkv_chunks(3, l, wb)
                emit_attn(1, inject=e3[0] + e3[1])
                def ln1_bcapply(seg, pre):
                    xf = [xF[c][seg][:, :] for c in range(NC_D)]
                    bc = ln_chain_bc(pre)
                    ln_apply(bc, xf,
                             (lambda c: ap_vec("G1", l * 4 + c)) if flags["g1"] else None,
                             (lambda c: ap_vec("BE1", l * 4 + c)) if flags["be1"] else None,
                             bf_out=[xT[c][seg][:, :] for c in range(NC_D)])

                emit_attn(2, inject=wo_chunks(0) + wo_chunks(1))
                b_stats, p_ln1 = {}, {}
                emit_attn(3, inject=wo_chunks(2))
                for f in wo_chunks(3):
                    f()

                def ffn_block(seg, defer_ln2=False):
                    xf = [xF[c][seg][:, :] for c in range(NC_D)]
                    xs = [xT[c][seg][:, :] for c in range(NC_D)]
                    for mc in range(NC_FF):
                        ps = P["ppro"].tile([128, TT], F32, tag="pp")
                        for kc in range(NC_D):
                            nc.tensor.matmul(
                                ps, w1_t[kc][:, mc * 128:(mc + 1) * 128], xs[kc],
                                start=(kc == 0), stop=(kc == NC_D - 1))
                        nc.scalar.activation(
                            hT[mc][:, :], ps, AF.Gelu,
                            bias=ap_vec("B1", l * 16 + mc) if flags["b1"] else 0.0,
                            scale=1.0)
                    # W2 in two waves of 2 output chunks (2 live accumulators)
                    sqts = []
                    for wave in range(2):
                        wps = [P["pacc"].tile([128, TT], F32, tag="pa",
                                              name=f"w2ps{m}") for m in range(2)]
                        for kc in range(NC_FF):
                            for m in range(2):
                                nc.tensor.matmul(
                                    wps[m],
                                    w2_t[kc][:, (wave * 2 + m) * 128:(wave * 2 + m + 1) * 128],
                                    hT[kc][:, :],
                                    start=(kc == 0), stop=(kc == NC_FF - 1))
                        for m in range(2):
                            mcD = wave * 2 + m
                            b_ap = ap_vec("B2", l * 4 + mcD) if flags["b2"] else None
                            nc.vector.scalar_tensor_tensor(
                                xf[mcD], wps[m], b_ap if b_ap is not None else 0.0,
                                xf[mcD], ALU.add, ALU.add)
                            sqts.append(mk_sq(xf[mcD]))
                    st = ln_stats(xf, D, sqts=sqts)
                    if defer_ln2:
                        pending_ln2.append((seg, st))
                        return
                    bc = ln_chain(st, D)
                    ln_apply(bc, xf,
                             (lambda c: ap_vec("G2", l * 4 + c)) if flags["g2"] else None,
                             (lambda c: ap_vec("BE2", l * 4 + c)) if flags["be2"] else None,
                             bf_out=xs)

                # ln1 chains split pre/bc: each pre runs under the stats
                # matmuls / ffn block emitted between it and its bc.
                b_stats[0] = wo_stats(0)
                b_stats[1] = wo_stats(1)
                p_ln1[0] = ln_chain_pre(b_stats[0], D)
                b_stats[2] = wo_stats(2)
                ln1_bcapply(0, p_ln1[0])
                p_ln1[1] = ln_chain_pre(b_stats[1], D)
                b_stats[3] = wo_stats(3)
                ln1_bcapply(1, p_ln1[1])
                p_ln1[2] = ln_chain_pre(b_stats[2], D)
                if not (emit_head and l == n_layers - 1):
                    ffn_block(0)
                    ln1_bcapply(2, p_ln1[2])
                    p_ln1[3] = ln_chain_pre(b_stats[3], D)
                    ffn_block(1)
                    ln1_bcapply(3, p_ln1[3])
                    ffn_block(2)
                    ffn_block(3, defer_ln2=True)
                else:
                    # last layer: wavefront the head stages through the FFN
                    # blocks. Every chain is split pre (DVE/ACT) / bc (PE)
                    # with PE-dense work emitted between them, and chains
                    # strictly alternate pre->bc so the bufs=1 mini slots
                    # never stall a pre on an unissued bc. Dummy keep-warm
                    # matmuls (kw) hold the HAM clock at 2.4GHz through the
                    # chain-latency-bound tail.
                    kw_ps = P["ppro"].tile([128, TT], F32, tag="pp", name="kwps")

                    def kw(n=3):
                        for _ in range(n):
                            nc.tensor.matmul(kw_ps, identb, xT[3][0][:, :],
                                             start=True, stop=True)

                    # segs {0,1} head pipelines entirely under ffn(3); segs
                    # {2,3} batch pairwise after it (m/yr bufs=2 lets two
                    # chain pres run back-to-back without waiting the first
                    # chain's broadcast matmuls).
                    s1, s2 = {}, {}
                    p1, p2, b1, b2k = {}, {}, {}, {}
                    ffn_block(0)
                    ln1_bcapply(2, p_ln1[2])
                    p_ln1[3] = ln_chain_pre(b_stats[3], D)
                    ffn_block(1)
                    ln1_bcapply(3, p_ln1[3])
                    ffn_block(2)
                    s1[0] = hp1(0); s1[1] = hp1(1)
                    p1[0] = ln_chain_pre(s1[0], D // 2)
                    p1[1] = ln_chain_pre(s1[1], D // 2)
                    ffn_block(3, defer_ln2=True)
                    (dseg, dst), = pending_ln2; pending_ln2.clear()
                    b1[0] = ln_chain_bc(p1[0]); apply1(0, b1[0])
                    b1[1] = ln_chain_bc(p1[1]); apply1(1, b1[1])
                    ln2p = ln_chain_pre(dst, D)
                    s2[0] = hp2(0); s2[1] = hp2(1)
                    ln2b = ln_chain_bc(ln2p)
                    ln_apply(ln2b, [xF[c][dseg][:, :] for c in range(NC_D)],
                             (lambda c: ap_vec("G2", l * 4 + c)) if flags["g2"] else None,
                             (lambda c: ap_vec("BE2", l * 4 + c)) if flags["be2"] else None,
                             bf_out=[xT[c][dseg][:, :] for c in range(NC_D)])
                    p2[0] = ln_chain_pre(s2[0], D // 4)
                    p2[1] = ln_chain_pre(s2[1], D // 4)
                    s1[2] = hp1(2)
                    b2k[0] = ln_chain_bc(p2[0]); apply2(0, b2k[0])
                    b2k[1] = ln_chain_bc(p2[1]); apply2(1, b2k[1])
                    s1[3] = hp1(3)
                    p1[2] = ln_chain_pre(s1[2], D // 2)
                    p1[3] = ln_chain_pre(s1[3], D // 2)
                    hp3(0); hp3(1)
                    b1[2] = ln_chain_bc(p1[2]); apply1(2, b1[2])
                    b1[3] = ln_chain_bc(p1[3]); apply1(3, b1[3])
                    kw(4)
                    s2[2] = hp2(2); s2[3] = hp2(3)
                    p2[2] = ln_chain_pre(s2[2], D // 4)
                    p2[3] = ln_chain_pre(s2[3], D // 4)
                    kw(4)
                    b2k[2] = ln_chain_bc(p2[2]); apply2(2, b2k[2])
                    b2k[3] = ln_chain_bc(p2[3]); apply2(3, b2k[3])
                    kw(4)
                    hp3(2); hp3(3)

            if not emit_head:
                flush_ln2(n_layers - 1)

            if dbg_x:
                for c in range(NC_D):
                    for s in range(NSEG):
                        nc.sync.dma_start(
                            out=xdbg_d[c * 128:(c + 1) * 128, s * TT:(s + 1) * TT],
                            in_=xT[c][s][:, :])

    nc.compile()
    return nc, extra


# ======================================================================
# Self-contained kernel entry point: takes FULL inputs, shards batch over
# 8 NeuronCores (data-parallel), runs the Bass kernel, gathers output.
# ======================================================================
from concourse.bass_utils import run_bass_kernel_spmd

N_CORES = 8


def make_in_maps(inputs, extra):
    base = dict(extra)
    obs = np.asarray(inputs["observations"], np.float32)
    n_b = obs.shape[0]
    per = n_b // N_CORES
    in_maps = []
    for c in range(N_CORES):
        m = dict(base)
        m["observations"] = np.ascontiguousarray(
            obs[c * per:(c + 1) * per]).astype(BF_NP)
        in_maps.append(m)
    return in_maps, per


def kernel(**inputs):
    inputs = {k: np.asarray(v) for k, v in inputs.items()}
    nc, extra = build(inputs, n_layers=8, emit_head=True, dbg_x=False)
    in_maps, per = make_in_maps(inputs, extra)

    last_err = None
    for attempt in range(4):
        try:
            res = run_bass_kernel_spmd(nc, in_maps, core_ids=list(range(N_CORES)),
                                       trace=False)
            outs = [res.results[c]["OUT"].reshape(per, S, ACT_DIM)
                    for c in range(N_CORES)]
            return np.concatenate(outs, axis=0)
        except Exception as e:  # transient NRT_EXEC_UNIT_UNRECOVERABLE etc.
            last_err = e
            import time as _time
            _time.sleep(3.0 * (attempt + 1))
    raise last_err

